# revision 1
# baseline (speedup 1.0000x reference)
"""Two-layer GCN (GCNConv x2 + ReLU) on 8 Trainium2 NeuronCores.

Strategy: partition nodes by destination across the 8 cores. Each core:
  1. computes the full H1 = X @ W1 table (replicated; avoids a collective),
  2. aggregates its 1/8 of destination nodes over its incident edges using
     one-hot matmuls accumulated in PSUM (exact fp32 scatter-add),
  3. AllGathers the layer-1 activations H2 across cores (split into row
     slices so layer 2 can start on early slices while later ones transfer),
  4. aggregates layer 2 the same way, then applies W2 + b2.
Edge gathers use the SWDGE dma_gather instruction (int16 indices). Node
feature tables are split into SLICES separate tensors of <=32k rows each so
indices fit int16; the host groups each dst-block's edges by source slice.
Tables and messages are bf16; all accumulation is fp32 in PSUM.
"""
import sys
sys.path.insert(0, '/opt/trn_rl_repo')
import numpy as np
import concourse.bass as bass
import concourse.bacc as bacc
import concourse.mybir as mybir
import bass_rust
from concourse.tile import TileContext
from concourse.tile_rust import add_dep_helper
from concourse.bass_utils import run_bass_kernel_spmd

dt = mybir.dt

NCORES = 8
SLICES = 4          # table row-slice count (separate tensors + AllGathers)
MAXG = 8            # SWDGE ring caps one dma_gather at 1024 indices
TAB_DT = dt.bfloat16   # table/message/one-hot dtype
XW_DT = dt.bfloat16    # X@W1 input dtype


def _np_dt(d):
    return mybir.dt.np(d)


# ---------------------------------------------------------------------------
# walrus in this toolchain rejects >1 attached sem wait on several opcodes;
# hoist extras into standalone InstEventSemaphore instructions just before.
def hoist_excess_waits(nc, max_attached=1):
    n_new = 0
    for f in nc.m.functions:
        for bb in f.blocks:
            insts = bb.instructions  # live list
            i = 0
            while i < len(insts):
                inst = insts[i]
                si = inst.sync_info
                if si is not None and inst.engine is not None:
                    waits = list(si.on_wait)
                    imm = [w for w in waits if w.wait_reg is None]
                    other = [w for w in waits if w.wait_reg is not None]
                    budget = max_attached - len(other)
                    if len(imm) > budget:
                        if budget > 0:
                            extra, keep = imm[:-budget], imm[-budget:]
                        else:
                            extra, keep = imm, []
                        for w in extra:
                            ev = mybir.InstEventSemaphore(
                                name=f"I-hoistw{n_new}", ins=[], outs=[])
                            ev.engine = inst.engine
                            h = bass_rust.SemaphoreHandle(name=w.ant_name, num=w.id)
                            bass_rust.wait_op(ev, h, w.wait_value, "sem-ge", True)
                            insts.insert(i, ev)
                            i += 1
                            n_new += 1
                        si.on_wait = other + keep
                i += 1
    return n_new


# ---------------------------------------------------------------------------
# host-side graph preprocessing
def _prepare(x, edge_index, ncores):
    N, D = x.shape
    src0 = edge_index[0].astype(np.int64)
    dst0 = edge_index[1].astype(np.int64)
    loops = np.arange(N, dtype=np.int64)
    src = np.concatenate([src0, loops])
    dst = np.concatenate([dst0, loops])

    deg = np.bincount(dst, minlength=N).astype(np.float32)
    dinv = 1.0 / np.sqrt(np.maximum(deg, 1.0))
    norm = (dinv[src] * dinv[dst]).astype(np.float32)

    NSH = (N + ncores - 1) // ncores            # nodes per shard (6250)
    TS = (NSH + 127) // 128                     # dst blocks per shard (49)
    # stage-group size: largest divisor of TS that is <= 8
    SG = max(s for s in range(1, 9) if TS % s == 0)
    NSHP = TS * 128                             # padded shard rows (6272)
    NPAD = ncores * NSHP
    # asymmetric slice bounds (shard rows): a smaller first slice lets its
    # AllGather start earlier; bounds are multiples of SG for the store split.
    GPR = NSHP // (SG * 128)
    if GPR >= SLICES:
        # distribute stage groups, extras to the earliest slices (their
        # AllGathers overlap layer-1 work; later ones gate the critical path)
        base, rem = GPR // SLICES, GPR % SLICES
        parts = [base + (1 if i < rem else 0) for i in range(SLICES)]
        BOUNDS = [0]
        for p in parts:
            BOUNDS.append(BOUNDS[-1] + p * SG * 128)
    else:
        step = max(SG, ((NSHP // SLICES) // SG) * SG)
        BOUNDS = [min(i * step, NSHP) for i in range(SLICES)] + [NSHP]
    RSLS = [BOUNDS[i + 1] - BOUNDS[i] for i in range(SLICES)]
    assert all(r > 0 and r % SG == 0 for r in RSLS)
    assert all(ncores * r <= 32768 for r in RSLS)

    # table row layout chosen so device-side stores are large contiguous DMAs:
    # node n -> shard c = n//NSH, shard-row r = g*(SG*128) + p*SG + s
    # (t = l//128 = g*SG+s, p = l%128); slice j = r//RSL holds table row
    # c*RSL + (r - j*RSL).
    def rowmap_shard(n):
        l = n % NSH
        t, p = l // 128, l % 128
        g, s = t // SG, t % SG
        return n // NSH, g * (SG * 128) + p * SG + s

    src_c, src_r = rowmap_shard(src)
    slice_flag = np.searchsorted(np.array(BOUNDS), src_r, side='right') - 1
    src_tab = (src_c * np.array(RSLS)[slice_flag]
               + (src_r - np.array(BOUNDS)[slice_flag]))

    dst_loc = dst % NSH
    dst_blk = dst_loc // 128
    dst_in_blk = dst_loc % 128
    dst_core = dst // NSH

    # per-(core, block, slice) edge lists
    groups = {}
    for c in range(ncores):
        eidx = np.nonzero(dst_core == c)[0]
        b_arr, j_arr = dst_blk[eidx], slice_flag[eidx]
        for b in range(TS):
            mb = b_arr == b
            for j in range(SLICES):
                groups[(c, b, j)] = eidx[mb & (j_arr == j)]

    # chunk counts per (block, slice): max over cores (shared program)
    m_cnt = {}
    for b in range(TS):
        for j in range(SLICES):
            mx = max(len(groups[(c, b, j)]) for c in range(ncores))
            m_cnt[(b, j)] = (mx + 127) // 128

    # flat chunk order: for G (super-group of SG blocks): for j: for b in G
    runs = []       # (j, [(b, m, chunk_off), ...])
    blk_first = {}
    blk_last = {}
    off = 0
    for G in range(TS // SG):
        for j in range(SLICES):
            blocks = []
            for b in range(G * SG, (G + 1) * SG):
                m = m_cnt[(b, j)]
                if m == 0:
                    continue
                for k in range(off, off + m):
                    if b not in blk_first:
                        blk_first[b] = k
                    blk_last[b] = k
                blocks.append((b, m, off))
                off += m
            if blocks:
                runs.append((j, blocks))
    NCHT = off

    # per-core flat edge arrays in chunk order, padded with null edges
    idx_np = np.zeros((ncores, 128, NCHT * 8), np.int16)
    dstl_np = np.zeros((ncores, 128, NCHT), np.float32)
    norm_np = np.zeros((ncores, 128, NCHT), np.float32)
    for c in range(ncores):
        flat_src = np.zeros(NCHT * 128, np.int64)
        flat_dst = np.zeros(NCHT * 128, np.float32)
        flat_nrm = np.zeros(NCHT * 128, np.float32)
        for (j, blocks) in runs:
            for (b, m, o) in blocks:
                e = groups[(c, b, j)]
                n = len(e)
                flat_src[o * 128:o * 128 + n] = src_tab[e]
                flat_dst[o * 128:o * 128 + n] = dst_in_blk[e]
                flat_nrm[o * 128:o * 128 + n] = norm[e]
        i16 = flat_src.astype(np.int16).reshape(-1, 16).T      # [16, NCHT*8]
        idx_np[c] = np.tile(i16, (8, 1))
        dstl_np[c] = flat_dst.reshape(NCHT, 128).T
        norm_np[c] = flat_nrm.reshape(NCHT, 128).T

    # xT columns in plain padded per-shard node order; the XW phase's staged
    # store applies the p/s permutation that lands rows at rowmap(n).
    nn = np.arange(N, dtype=np.int64)
    colmap = (nn // NSH) * NSHP + (nn % NSH)
    xT = np.zeros((D, NPAD), np.float32)
    xT[:, colmap] = x.T

    iota = np.tile(np.arange(128, dtype=np.float32)[None, :], (128, 1)).copy()

    return dict(N=N, D=D, NSH=NSH, TS=TS, SG=SG, NSHP=NSHP, NPAD=NPAD,
                BOUNDS=BOUNDS, RSLS=RSLS, NCHT=NCHT, runs=runs,
                blk_first=blk_first, blk_last=blk_last,
                idx_np=idx_np, dstl_np=dstl_np, norm_np=norm_np,
                xT=xT, iota=iota)


# ---------------------------------------------------------------------------
def _build(cfg, F1, F2, debug=False):
    NPAD, NSHP = cfg['NPAD'], cfg['NSHP']
    BOUNDS, RSLS = cfg['BOUNDS'], cfg['RSLS']
    D, TS, SG, NCHT = cfg['D'], cfg['TS'], cfg['SG'], cfg['NCHT']
    runs, blk_first, blk_last = cfg['runs'], cfg['blk_first'], cfg['blk_last']
    KD = D // 128
    GROUPS = NPAD // (SG * 128)     # XW stage groups over the whole table
    SGROWS = SG * 128
    GPS = GROUPS // NCORES          # XW stage groups per shard (= TS//SG)

    nc = bacc.Bacc(None, target_bir_lowering=False)
    xT_d = nc.declare_dram_parameter("xT", [D, NPAD], XW_DT, isOutput=False)
    W1_d = nc.declare_dram_parameter("W1", [D, F1], XW_DT, isOutput=False)
    b1_d = nc.declare_dram_parameter("b1", [F1, 1], dt.float32, isOutput=False)
    W2_d = nc.declare_dram_parameter("W2", [F1, F2], TAB_DT, isOutput=False)
    b2_d = nc.declare_dram_parameter("b2", [F2, 1], dt.float32, isOutput=False)
    iota_d = nc.declare_dram_parameter("iota", [128, 128], TAB_DT, isOutput=False)
    ones1_d = nc.declare_dram_parameter("ones1", [1, 128], dt.float32, isOutput=False)
    idx_d = nc.declare_dram_parameter("idx", [128, NCHT * 8], dt.int16, isOutput=False)
    dstl_d = nc.declare_dram_parameter("dstl", [128, NCHT], dt.float32, isOutput=False)
    norm_d = nc.declare_dram_parameter("norm", [128, NCHT], dt.float32, isOutput=False)
    out_d = nc.declare_dram_parameter("outT", [F2, NSHP], dt.float32, isOutput=True)

    # per-slice node-feature tables, [8*RSL, F1] each (row = c*RSL + r)
    H1tabs = [nc.dram_tensor(f"H1tab{j}", [NCORES * RSLS[j], F1], TAB_DT)
              for j in range(SLICES)]
    H2shs = [nc.dram_tensor(f"H2sh{j}", [RSLS[j], F1], TAB_DT)
             for j in range(SLICES)]
    H2tabs = [nc.dram_tensor(f"H2tab{j}", [NCORES * RSLS[j], F1], TAB_DT,
                             addr_space="Shared") for j in range(SLICES)]
    if debug:
        H1dbg = nc.declare_dram_parameter("H1dbg", [NPAD, F1], TAB_DT, isOutput=True)
        H2dbg = nc.declare_dram_parameter("H2dbg", [NPAD, F1], TAB_DT, isOutput=True)

    with TileContext(nc) as tc:
        with (
            tc.tile_pool(name="const", bufs=1) as cp,
            tc.tile_pool(name="xw", bufs=3) as xp,
            tc.tile_pool(name="gat", bufs=4) as gp,
            tc.tile_pool(name="oh", bufs=6) as ohp,
            tc.tile_pool(name="evac", bufs=3) as evp,
        ):
            # ---- constants / metadata resident in SBUF ----
            iota_t = cp.tile([128, 128], TAB_DT, tag="iota")
            nc.sync.dma_start(iota_t[:], iota_d[:])
            ones1_t = cp.tile([1, 128], dt.float32, tag="ones1")
            nc.sync.dma_start(ones1_t[:], ones1_d[:])
            b1r_t = cp.tile([1, F1], dt.float32, tag="b1r")
            nc.sync.dma_start(b1r_t[:], b1_d[:].rearrange("f one -> one f"))
            b2_t = cp.tile([F2, 1], dt.float32, tag="b2")
            nc.sync.dma_start(b2_t[:], b2_d[:])
            W1_t = cp.tile([D if KD == 1 else 128, KD, F1], XW_DT, tag="W1")
            nc.sync.dma_start(W1_t[:], W1_d[:].rearrange("(k p) f -> p k f", p=128 if KD > 1 else D))
            W2_t = cp.tile([F1, F2], TAB_DT, tag="W2")
            nc.sync.dma_start(W2_t[:], W2_d[:])
            idx_t = cp.tile([128, NCHT * 8], dt.int16, tag="idx")
            nc.sync.dma_start(idx_t[:], idx_d[:])
            dstl_t = cp.tile([128, NCHT], dt.float32, tag="dstl")
            nc.sync.dma_start(dstl_t[:], dstl_d[:])
            norm_t = cp.tile([128, NCHT], dt.float32, tag="norm")
            nc.sync.dma_start(norm_t[:], norm_d[:])

            def store_group_rows(tensors, stage, c, g_in_shard, nrows_per_p):
                """DMA a staged [P, nrows_per_p, F] tile into the per-slice
                tables. Shard rows covered: r0 + p*nrows_per_p + s. Split on
                the partition dim at slice boundaries. Returns [(j, inst)].
                `tensors[j]` row base is c*RSL (c=None for shard-local)."""
                r0 = g_in_shard * 128 * nrows_per_p
                out = []
                for j in range(SLICES):
                    p0 = max(0, (BOUNDS[j] - r0)) // nrows_per_p
                    p1 = min(128, max(0, BOUNDS[j + 1] - r0) // nrows_per_p)
                    if p1 <= p0:
                        continue
                    base = (0 if c is None else c * RSLS[j]) + r0 - BOUNDS[j]
                    w = nc.sync.dma_start(
                        tensors[j][base + p0 * nrows_per_p:
                                   base + p1 * nrows_per_p, :]
                        .rearrange("(p s) f -> p s f", s=nrows_per_p),
                        stage[p0:p1, :, :])
                    out.append((j, w))
                return out

            # ---- phase 1: H1 = X @ W1, full table, node(-row)-major ----
            # emit slice-major so layer-1 slice-j gathers unblock early
            xw_writes = {j: [] for j in range(SLICES)}
            xw_order = []
            for jj in range(SLICES):
                for gg in range(GPS):
                    if max(i for i in range(SLICES)
                           if BOUNDS[i] <= gg * SGROWS) == jj:
                        for c in range(NCORES):
                            xw_order.append(c * GPS + gg)
            assert sorted(xw_order) == list(range(GROUPS))
            with tc.tile_pool(name="xwps", bufs=4, space="PSUM") as xpp:
                for g in xw_order:
                    xt = xp.tile([128, KD, SGROWS], XW_DT, tag="xt")
                    nc.sync.dma_start(
                        xt[:],
                        xT_d[:, g * SGROWS:(g + 1) * SGROWS].rearrange(
                            "(k p) n -> p k n", p=128 if KD > 1 else D))
                    stage = xp.tile([128, SG, F1], TAB_DT, tag="h1stage")
                    for s in range(SG):
                        ps = xpp.tile([128, F1], dt.float32, tag="xwps")
                        for k in range(KD):
                            nc.tensor.matmul(ps[:], xt[:, k, s * 128:(s + 1) * 128],
                                             W1_t[:, k, :],
                                             start=(k == 0), stop=(k == KD - 1))
                        nc.vector.tensor_copy(stage[:, s, :], ps[:])
                    for (j, w) in store_group_rows(H1tabs, stage,
                                                   g // GPS, g % GPS, SG):
                        xw_writes[j].append(w)

            # ---- shared aggregation emitter over the chunk schedule ----
            # node_major=True  -> acc[dst, f]  (lhsT=onehot, rhs=msgs)
            # node_major=False -> acc[f, dst]  (lhsT=msgs, rhs=onehot)
            def agg_layer(tabs, node_major, gather_deps, psum_pool,
                          after_group=None, extra_mms=0, runs_subset=None,
                          acc_tag="acc"):
                my_runs = runs if runs_subset is None else runs_subset
                last_of_blk = {}
                for (j, blocks) in my_runs:
                    for (b, m, o) in blocks:
                        last_of_blk[b] = o + m - 1
                accs = {}
                done_in_blk = {}

                def get_acc(b):
                    if b not in accs:
                        shape = [128, F1] if node_major else [F1, 128]
                        accs[b] = psum_pool.tile(shape, dt.float32,
                                                 name=f"{acc_tag}{b}", tag=acc_tag)
                        done_in_blk[b] = 0
                    return accs[b]

                cur_G = my_runs[0][1][0][0] // SG
                for (j, blocks) in my_runs:
                    G = blocks[0][0] // SG
                    if G != cur_G:
                        if after_group is not None:
                            after_group(cur_G, accs)
                        cur_G = G
                    chunk_list = [(b, k) for (b, m, o) in blocks
                                  for k in range(o, o + m)]
                    for w0 in range(0, len(chunk_list), MAXG):
                        win = chunk_list[w0:w0 + MAXG]
                        o = win[0][1]
                        m = len(win)
                        gt = gp.tile([128, MAXG, F1], TAB_DT, tag="gat")
                        gi = nc.gpsimd.dma_gather(
                            gt[:, 0:m, :], tabs[j][:], idx_t[:, o * 8:(o + m) * 8],
                            num_idxs=m * 128, num_idxs_reg=m * 128, elem_size=F1)
                        for dep in gather_deps[j]:
                            add_dep_helper(gi.ins, dep.ins, reason="gather table dep")
                        for slot, (b, k) in enumerate(win):
                            acc = get_acc(b)
                            oh = ohp.tile([128, 128], TAB_DT, tag="oh")
                            nc.vector.tensor_scalar(
                                oh[:], iota_t[:], dstl_t[:, k:k + 1],
                                norm_t[:, k:k + 1],
                                mybir.AluOpType.is_equal, mybir.AluOpType.mult)
                            first = (done_in_blk[b] == 0)
                            done_in_blk[b] += 1
                            last = (k == last_of_blk[b]) and extra_mms == 0
                            if node_major:
                                nc.tensor.matmul(acc[:], oh[:], gt[:, slot, :],
                                                 start=first, stop=last)
                            else:
                                nc.tensor.matmul(acc[:], gt[:, slot, :], oh[:],
                                                 start=first, stop=last)
                if after_group is not None:
                    after_group(cur_G, accs)
                return accs

            # ---- layer 1: aggregate (node-major), +b1 via K=1 matmul, relu ----
            h2_writes = {j: [] for j in range(SLICES)}
            ag_deps = {}

            def l1_after_group(g, accs1):
                stage = evp.tile([128, SG, F1], TAB_DT, tag="h2stage")
                for s in range(SG):
                    b = g * SG + s
                    nc.tensor.matmul(accs1[b][:], ones1_t[:], b1r_t[:],
                                     start=False, stop=True)
                    nc.scalar.activation(stage[:, s, :], accs1[b][:],
                                         mybir.ActivationFunctionType.Relu,
                                         bias=0.0, scale=1.0)
                for (j, w) in store_group_rows(H2shs, stage, None, g, SG):
                    h2_writes[j].append(w)
                # AllGather slice j right after its last stage group's write,
                # so it sits early in the Pool queue and overlaps the rest of
                # layer 1 on the collective cores.
                for j in range(SLICES):
                    if g == (BOUNDS[j + 1] - 1) // SGROWS:
                        cc = nc.gpsimd.collective_compute(
                            "AllGather", mybir.AluOpType.bypass,
                            replica_groups=[list(range(NCORES))],
                            ins=[H2shs[j][:]], outs=[H2tabs[j][:]])
                        for w in h2_writes[j]:
                            add_dep_helper(cc.ins, w.ins,
                                           reason="allgather reads H2 slice")
                        ag_deps[j] = [cc]

            with tc.tile_pool(name="aggps1", bufs=SG, space="PSUM") as app1:
                agg_layer(H1tabs, True, xw_writes, app1,
                          after_group=l1_after_group, extra_mms=1)


            # ---- layer 2: aggregate H2 (feature-major), then W2 + b2 ----
            with (
                tc.tile_pool(name="aggps2", bufs=SG, space="PSUM") as app2,
                tc.tile_pool(name="w2ps", bufs=1, space="PSUM") as wpp,
                tc.tile_pool(name="part2", bufs=2) as p2pool,
            ):
                # N-pass layer 2: pass si aggregates slice-si edges into
                # PSUM; non-final passes park/merge partials in SBUF so the
                # work overlaps the later slices' AllGathers; the final pass
                # merges and applies W2 + b2.
                part2 = {}

                def l2_mid_after(g, accs2):
                    for s in range(SG):
                        b = g * SG + s
                        if b not in accs2:
                            continue
                        pt = p2pool.tile([F1, 128], TAB_DT,
                                         name=f"part2_{b}", tag=f"p2_{b}")
                        if b in part2:
                            nc.vector.tensor_tensor(
                                pt[:], accs2[b][:], part2[b][:],
                                mybir.AluOpType.add)
                        else:
                            nc.scalar.activation(pt[:], accs2[b][:],
                                                 mybir.ActivationFunctionType.Copy)
                        part2[b] = pt

                def l2_final_after(g, accs2):
                    ostage = evp.tile([F2, SG, 128], dt.float32, tag="ostage")
                    for s in range(SG):
                        b = g * SG + s
                        m2 = evp.tile([F1, 128], TAB_DT, name=f"m2_{b}", tag="m2")
                        if b in accs2 and b in part2:
                            nc.vector.tensor_tensor(
                                m2[:], accs2[b][:], part2[b][:],
                                mybir.AluOpType.add)
                        elif b in accs2:
                            nc.scalar.activation(m2[:], accs2[b][:],
                                                 mybir.ActivationFunctionType.Copy)
                        else:
                            nc.vector.tensor_copy(m2[:], part2[b][:])
                        p2 = wpp.tile([F2, 128], dt.float32, tag="w2ps")
                        nc.tensor.matmul(p2[:], W2_t[:], m2[:],
                                         start=True, stop=True)
                        nc.scalar.activation(ostage[:, s, :], p2[:],
                                             mybir.ActivationFunctionType.Identity,
                                             bias=b2_t[:, 0:1], scale=1.0)
                    nc.sync.dma_start(
                        out_d[:, g * SGROWS:(g + 1) * SGROWS].rearrange(
                            "f (s n) -> f s n", s=SG),
                        ostage[:])

                for si in range(SLICES):
                    runs_si = [r for r in runs if r[0] == si]
                    agg_layer(H2tabs, False, ag_deps, app2,
                              after_group=(l2_final_after if si == SLICES - 1
                                           else l2_mid_after),
                              runs_subset=runs_si)

                if debug:
                    tc.strict_bb_all_engine_barrier()
                    for j in range(SLICES):
                        for c in range(NCORES):
                            r0 = c * NSHP + BOUNDS[j]
                            nc.sync.dma_start(
                                H1dbg[r0:r0 + RSLS[j], :],
                                H1tabs[j][c * RSLS[j]:(c + 1) * RSLS[j], :])
                            nc.sync.dma_start(
                                H2dbg[r0:r0 + RSLS[j], :],
                                H2tabs[j][c * RSLS[j]:(c + 1) * RSLS[j], :])

    if not nc.is_finalized():
        nc.finalize()
    hoist_excess_waits(nc)
    return nc


# ---------------------------------------------------------------------------
def _kernel_impl(x, edge_index, W1, b1, W2, b2, ncores=NCORES, debug=False):
    x = np.asarray(x, dtype=np.float32)
    edge_index = np.asarray(edge_index)
    W1 = np.asarray(W1, dtype=np.float32)
    b1 = np.asarray(b1, dtype=np.float32)
    W2 = np.asarray(W2, dtype=np.float32)
    b2 = np.asarray(b2, dtype=np.float32)
    N, D = x.shape
    F1 = W1.shape[1]
    F2 = W2.shape[1]

    cfg = _prepare(x, edge_index, ncores)
    nc = _build(cfg, F1, F2, debug=debug)

    xwnp = _np_dt(XW_DT)
    tabnp = _np_dt(TAB_DT)
    in_maps = []
    for c in range(ncores):
        in_maps.append({
            "xT": cfg['xT'].astype(xwnp),
            "W1": W1.astype(xwnp),
            "b1": b1.reshape(F1, 1).astype(np.float32),
            "W2": W2.astype(tabnp),
            "b2": b2.reshape(F2, 1).astype(np.float32),
            "iota": cfg['iota'].astype(tabnp),
            "ones1": np.ones((1, 128), np.float32),
            "idx": cfg['idx_np'][c],
            "dstl": cfg['dstl_np'][c],
            "norm": cfg['norm_np'][c],
        })
    res = run_bass_kernel_spmd(nc, in_maps, list(range(ncores)))

    NSH = cfg['NSH']
    out = np.empty((N, F2), np.float32)
    # outT columns are plain shard-local node order (col = t*128 + p = l)
    for c in range(ncores):
        oT = res.results[c]["outT"]          # [F2, NSHP]
        n0 = c * NSH
        n1 = min(N, n0 + NSH)
        out[n0:n1] = oT[:, :n1 - n0].T
    return out, res, nc, cfg


def kernel(x, edge_index, W1, b1, W2, b2):
    out, _, _, _ = _kernel_impl(x, edge_index, W1, b1, W2, b2)
    return out



# revision 3
# speedup vs baseline: 1.4370x; 1.4370x over previous
"""Two-layer GCN (GCNConv x2 + ReLU) on 8 Trainium2 NeuronCores.

Strategy (aggregate-first, dinv-folded, fp8 tables):
  - Nodes sharded by destination across 8 cores. Layer 1 aggregates RAW
    input features: each core gathers X'[src] rows per edge (X' = dinv*X,
    host-prescaled, fp8, laid out in 2 slice tables), scatter-adds them into
    per-dst-block PSUM accumulators (feature-major) via pure 0/1 one-hot
    matmuls, then applies W1 + bias + ReLU + W2 per 128-node block on-chip.
    No X@W1 table phase at all.
  - dinv normalization folded exactly: one-hots are pure 0/1; source dinv
    lives in the tables; dst dinv is applied as a per-partition activation
    scale (relu is positively homogeneous): H3' table gets dinv^2, final
    output gets dinv. Biases enter as K=1 matmuls of b (x) sqrt(deg).
  - H3' = dinv^2*relu(AGG@W1 + b1*sqrt(deg))@W2 stored fp8 e4m3 at 64
    features -> the AllGather moves 3.2MB total instead of 12.8MB. After the
    AllGather each slice is expanded to 256B-row stride (dma_gather needs
    elem/stride multiples of 256B); layer 2 gathers those rows node-major.
  - One-hots are built once per 128-edge chunk as packed uint16 pairs on DVE
    (2-byte dtype keeps the 4x DVE mode, 77ns) and bitcast to fp8 [128,128].
  - Self-loops are excluded from the edge lists (that removes exactly one
    full chunk per (block, slice-of-own-rows)): their contribution enters
    via identity matmuls against directly-read own-shard X' rows (layer 1)
    and the layer-1 H3 evac tiles still in SBUF (layer 2).
  - Edge chunks grouped (supergroup of 7 dst blocks) x (source slice of 2)
    so one SWDGE dma_gather covers up to 4096 edges (scratch ring enlarged),
    amortizing the ~1.1us per-gather prep cost. Chunks are CONSUMED
    block-major so only ~2 blocks' PSUM accumulators are live.
"""
import sys
sys.path.insert(0, '/opt/trn_rl_repo')
import numpy as np
import concourse.bass as bass
import concourse.bacc as bacc
import concourse.mybir as mybir
import bass_rust
from concourse.tile import TileContext
from concourse.tile_rust import add_dep_helper
from concourse.bass_utils import run_bass_kernel_spmd

dt = mybir.dt

NCORES = 8
SLICES = 2
WCHUNK = 32            # chunks per gather window (32*128 = 4096 indices)
SCRATCH = 65536        # SWDGE ring: 4096 descriptors
TAB8 = dt.float8e4     # table dtype (e4m3)
PAD_DST = 999.0        # one-hot miss value for padding edges


def _np_dt(d):
    return mybir.dt.np(d)


# ---------------------------------------------------------------------------
# walrus in this toolchain rejects >1 attached sem wait on several opcodes;
# hoist extras into standalone InstEventSemaphore instructions just before.
def hoist_excess_waits(nc, max_attached=1):
    n_new = 0
    for f in nc.m.functions:
        for bb in f.blocks:
            insts = bb.instructions  # live list
            i = 0
            while i < len(insts):
                inst = insts[i]
                si = inst.sync_info
                if si is not None and inst.engine is not None:
                    waits = list(si.on_wait)
                    imm = [w for w in waits if w.wait_reg is None]
                    other = [w for w in waits if w.wait_reg is not None]
                    budget = max_attached - len(other)
                    if len(imm) > budget:
                        if budget > 0:
                            extra, keep = imm[:-budget], imm[-budget:]
                        else:
                            extra, keep = imm, []
                        for w in extra:
                            ev = mybir.InstEventSemaphore(
                                name=f"I-hoistw{n_new}", ins=[], outs=[])
                            ev.engine = inst.engine
                            h = bass_rust.SemaphoreHandle(name=w.ant_name, num=w.id)
                            bass_rust.wait_op(ev, h, w.wait_value, "sem-ge", True)
                            insts.insert(i, ev)
                            i += 1
                            n_new += 1
                        si.on_wait = other + keep
                i += 1
    return n_new


# ---------------------------------------------------------------------------
# host-side graph preprocessing
def _prepare(x, edge_index, ncores):
    x = np.asarray(x, dtype=np.float32)
    N, D = x.shape
    NSH = (N + ncores - 1) // ncores            # nodes per shard (6250)
    TS = (NSH + 127) // 128                     # dst blocks per shard (49)
    GS = max(s for s in range(1, 9) if TS % s == 0)   # blocks per supergroup
    NG = TS // GS                               # supergroups per shard
    NSHP = TS * 128                             # padded shard rows

    # slice bounds in shard rows (multiples of GS*128); near-even split
    gb, rem = NG // SLICES, NG % SLICES
    parts = [gb + (1 if i < rem else 0) for i in range(SLICES)]
    BOUNDS = [0]
    for p in parts:
        BOUNDS.append(BOUNDS[-1] + p * GS * 128)
    RSLS = [BOUNDS[i + 1] - BOUNDS[i] for i in range(SLICES)]
    assert all(ncores * r < 32768 for r in RSLS), RSLS
    BARR = np.array(BOUNDS)
    RARR = np.array(RSLS)

    src = edge_index[0].astype(np.int64)        # self-loops handled separately
    dst = edge_index[1].astype(np.int64)
    E = len(src)

    deg = np.bincount(dst, minlength=N).astype(np.float32) + 1.0  # + self loop
    dinv = 1.0 / np.sqrt(deg)

    # source table row (slice-local): node s -> shard c, local l;
    # slice j of l; row = c*RSL_j + (l - B_j)
    src_c, src_l = src // NSH, src % NSH
    src_j = (np.searchsorted(BARR, src_l, side='right') - 1).astype(np.int64)
    src_row = src_c * RARR[src_j] + (src_l - BARR[src_j])

    dst_c, dst_l = dst // NSH, dst % NSH
    dst_b = dst_l // 128
    dst_p = dst_l % 128

    # cell = (core, block, slice); shared chunk schedule = max count per cell
    cell = (dst_c * TS + dst_b) * SLICES + src_j
    counts = np.bincount(cell, minlength=ncores * TS * SLICES)
    counts3 = counts.reshape(ncores, TS, SLICES)
    m_cnt = (counts3.max(axis=0) + 127) // 128          # [TS, SLICES]

    # chunk offsets in gather order: for G: for j: for b in G
    off = np.zeros((TS, SLICES), np.int64)
    chunk_block = []
    runs = {}            # (G, j) -> (k_start, k_end)
    k = 0
    for G in range(NG):
        for j in range(SLICES):
            k0 = k
            for b in range(G * GS, (G + 1) * GS):
                off[b, j] = k
                m = int(m_cnt[b, j])
                chunk_block.extend([b] * m)
                k += m
            runs[(G, j)] = (k0, k)
    NCHT = k
    chunk_block = np.array(chunk_block, np.int64)

    # rank of each edge within its cell (stable order)
    order = np.argsort(cell, kind='stable')
    starts = np.zeros(ncores * TS * SLICES + 1, np.int64)
    starts[1:] = np.cumsum(counts)
    rank = np.empty(E, np.int64)
    rank[order] = np.arange(E) - starts[cell[order]]

    slot = off[dst_b, src_j] * 128 + rank       # flat slot per edge

    idx_np = np.zeros((ncores, 128, NCHT * 8), np.int16)
    dpair_np = np.full((ncores, 128, NCHT), PAD_DST, np.float32)
    dpar_np = np.zeros((ncores, 128, NCHT), np.float32)
    for c in range(ncores):
        m = dst_c == c
        fsrc = np.zeros(NCHT * 128, np.int64)
        fdst = np.full(NCHT * 128, PAD_DST, np.float32)
        fsrc[slot[m]] = src_row[m]
        fdst[slot[m]] = dst_p[m]
        i16 = fsrc.astype(np.int16).reshape(-1, 16).T      # [16, NCHT*8]
        idx_np[c] = np.tile(i16, (8, 1))
        pair = np.where(fdst == PAD_DST, PAD_DST, np.floor(fdst / 2.0))
        par = np.where(fdst % 2.0 == 0.0, 56.0, 14336.0)   # fp8 1.0 lo/hi byte
        dpair_np[c] = pair.reshape(NCHT, 128).T
        dpar_np[c] = par.reshape(NCHT, 128).T

    # X' tables per slice (fp8), row = c*RSL_j + (l - B_j); pad rows zero
    f8 = _np_dt(TAB8)
    xp = (x * dinv[:, None]).astype(np.float32)
    Xs = []
    for j in range(SLICES):
        t = np.zeros((ncores * RSLS[j], D), np.float32)
        for c in range(ncores):
            l0, l1 = BOUNDS[j], BOUNDS[j + 1]
            n0 = c * NSH + l0
            n1 = min(c * NSH + min(l1, NSH), N)
            if n1 > n0:
                t[c * RSLS[j]:c * RSLS[j] + (n1 - n0)] = xp[n0:n1]
        Xs.append(t.astype(f8))

    # per-core own-shard X' rows (plain order) for the self-loop term
    Xown_np = np.zeros((ncores, NSHP, D), np.float32)
    for c in range(ncores):
        n0, n1 = c * NSH, min((c + 1) * NSH, N)
        Xown_np[c, :n1 - n0] = xp[n0:n1]
    Xown_np = Xown_np.astype(f8)

    # per-core dst-side scales
    dinv2_np = np.ones((ncores, 128, TS), np.float32)
    dinv1_np = np.ones((ncores, 128, TS), np.float32)
    recip_np = np.zeros((ncores, 1, NSHP), np.float32)
    for c in range(ncores):
        n0, n1 = c * NSH, min((c + 1) * NSH, N)
        dloc = np.ones(NSHP, np.float32)
        dloc[:n1 - n0] = dinv[n0:n1]
        dinv2_np[c] = (dloc ** 2).reshape(TS, 128).T
        dinv1_np[c] = dloc.reshape(TS, 128).T
        r = np.zeros(NSHP, np.float32)
        r[:n1 - n0] = 1.0 / dinv[n0:n1]
        recip_np[c, 0] = r

    iota64 = np.tile(np.arange(64, dtype=np.uint16)[None, :], (128, 1)).copy()
    id128 = np.eye(128, dtype=np.float32).astype(f8)

    return dict(N=N, D=D, NSH=NSH, TS=TS, GS=GS, NG=NG, NSHP=NSHP,
                BOUNDS=BOUNDS, RSLS=RSLS, NCHT=NCHT, runs=runs,
                m_cnt=m_cnt, off=off, chunk_block=chunk_block,
                idx_np=idx_np, dpair_np=dpair_np, dpar_np=dpar_np,
                Xs=Xs, Xown_np=Xown_np, dinv2_np=dinv2_np,
                dinv1_np=dinv1_np, recip_np=recip_np,
                iota64=iota64, id128=id128)


# ---------------------------------------------------------------------------
def _build(cfg, F1, F2):
    D, TS, GS, NG = cfg['D'], cfg['TS'], cfg['GS'], cfg['NG']
    NSHP, NCHT = cfg['NSHP'], cfg['NCHT']
    BOUNDS, RSLS = cfg['BOUNDS'], cfg['RSLS']
    runs, m_cnt, off = cfg['runs'], cfg['m_cnt'], cfg['off']
    KD = D // 128

    nc = bacc.Bacc(None, target_bir_lowering=False,
                   dynamic_dma_scratch_size=SCRATCH)
    Xs_d = [nc.declare_dram_parameter(f"Xs{j}", [NCORES * RSLS[j], D],
                                      TAB8, isOutput=False)
            for j in range(SLICES)]
    Xown_d = nc.declare_dram_parameter("Xown", [NSHP, D], TAB8, isOutput=False)
    W1_d = nc.declare_dram_parameter("W1", [D, F1], dt.bfloat16, isOutput=False)
    W2_d = nc.declare_dram_parameter("W2", [F1, F2], dt.bfloat16, isOutput=False)
    b1_d = nc.declare_dram_parameter("b1", [1, F1], dt.bfloat16, isOutput=False)
    b2_d = nc.declare_dram_parameter("b2", [1, F2], dt.bfloat16, isOutput=False)
    iota64_d = nc.declare_dram_parameter("iota64", [128, 64], dt.uint16, isOutput=False)
    id128_d = nc.declare_dram_parameter("id128", [128, 128], TAB8, isOutput=False)
    idx_d = nc.declare_dram_parameter("idx", [128, NCHT * 8], dt.int16, isOutput=False)
    dpair_d = nc.declare_dram_parameter("dpair", [128, NCHT], dt.float32, isOutput=False)
    dpar_d = nc.declare_dram_parameter("dpar", [128, NCHT], dt.float32, isOutput=False)
    dinv2_d = nc.declare_dram_parameter("dinv2", [128, TS], dt.float32, isOutput=False)
    dinv1_d = nc.declare_dram_parameter("dinv1", [128, TS], dt.float32, isOutput=False)
    recip_d = nc.declare_dram_parameter("recip", [1, NSHP], dt.bfloat16, isOutput=False)
    out_d = nc.declare_dram_parameter("out", [NSHP, F2], dt.float32, isOutput=True)

    H3shs = [nc.dram_tensor(f"H3sh{j}", [RSLS[j], F2], TAB8)
             for j in range(SLICES)]
    H3tabs = [nc.dram_tensor(f"H3tab{j}", [NCORES * RSLS[j], F2], TAB8,
                             addr_space="Shared") for j in range(SLICES)]
    H3exp = [nc.dram_tensor(f"H3exp{j}", [NCORES * RSLS[j], 256], TAB8)
             for j in range(SLICES)]

    def blk_slice(b):
        return 0 if (b + 1) * 128 <= BOUNDS[1] else 1

    with TileContext(nc) as tc:
        with (
            tc.tile_pool(name="const", bufs=1) as cp,
            tc.tile_pool(name="l1gt", bufs=6) as g1p,
            tc.tile_pool(name="l2gt", bufs=4) as g2p,
            tc.tile_pool(name="oh16", bufs=10) as ohp,
            tc.tile_pool(name="evac", bufs=3) as evp,
            tc.tile_pool(name="h3sb", bufs=1) as h3p,
            tc.tile_pool(name="park", bufs=1) as pkp,
        ):
            # ---- constants / metadata resident in SBUF ----
            iota64_t = cp.tile([128, 64], dt.uint16, tag="iota64")
            nc.sync.dma_start(iota64_t[:], iota64_d[:])
            id_t = cp.tile([128, 128], TAB8, tag="id128")
            nc.sync.dma_start(id_t[:], id128_d[:])
            W1_t = cp.tile([128, KD, F1], dt.bfloat16, tag="W1")
            nc.sync.dma_start(W1_t[:], W1_d[:].rearrange("(k p) f -> p k f", p=128))
            W2_t = cp.tile([F1, F2], dt.bfloat16, tag="W2")
            nc.sync.dma_start(W2_t[:], W2_d[:])
            b1_t = cp.tile([1, F1], dt.bfloat16, tag="b1")
            nc.sync.dma_start(b1_t[:], b1_d[:])
            b2_t = cp.tile([1, F2], dt.bfloat16, tag="b2")
            nc.sync.dma_start(b2_t[:], b2_d[:])
            xo_t = cp.tile([128, TS, KD, 128], TAB8, tag="Xown")
            nc.sync.dma_start(
                xo_t[:], Xown_d[:].rearrange("(t p) (k f) -> p t k f",
                                             p=128, k=KD))
            idx_t = cp.tile([128, NCHT * 8], dt.int16, tag="idx")
            nc.sync.dma_start(idx_t[:], idx_d[:])
            dpair_t = cp.tile([128, NCHT], dt.float32, tag="dpair")
            nc.sync.dma_start(dpair_t[:], dpair_d[:])
            dpar_t = cp.tile([128, NCHT], dt.float32, tag="dpar")
            nc.sync.dma_start(dpar_t[:], dpar_d[:])
            dinv2_t = cp.tile([128, TS], dt.float32, tag="dinv2")
            nc.sync.dma_start(dinv2_t[:], dinv2_d[:])
            dinv1_t = cp.tile([128, TS], dt.float32, tag="dinv1")
            nc.sync.dma_start(dinv1_t[:], dinv1_d[:])
            recip_t = cp.tile([1, NSHP], dt.bfloat16, tag="recip")
            nc.sync.dma_start(recip_t[:], recip_d[:])

            def make_oh(kk):
                oh = ohp.tile([128, 64], dt.uint16, tag="oh16")
                nc.vector.tensor_scalar(
                    oh[:], iota64_t[:], dpair_t[:, kk:kk + 1],
                    dpar_t[:, kk:kk + 1],
                    mybir.AluOpType.is_equal, mybir.AluOpType.mult)
                return oh[:].bitcast(TAB8)

            def win_tile_slot(G, j, kk, tiles):
                """gather tile + slot for chunk kk of run (G, j)."""
                k0, _ = runs[(G, j)]
                w = (kk - k0) // WCHUNK
                return tiles[(G, j)][w], (kk - k0) % WCHUNK

            h3_writes = {j: [] for j in range(SLICES)}
            h3s_tiles = {}
            exps = {}

            # ================= layer 1 =================
            with (
                tc.tile_pool(name="l1ps", bufs=2, space="PSUM") as app1,
                tc.tile_pool(name="h2ps", bufs=1, space="PSUM") as hpp,
                tc.tile_pool(name="h3ps", bufs=1, space="PSUM") as tpp,
            ):
                gt_tiles = {}
                for G in range(NG):
                    # issue all gathers of this supergroup (both slices)
                    for j in range(SLICES):
                        k0, k1 = runs[(G, j)]
                        tiles = []
                        for o in range(k0, k1, WCHUNK):
                            m = min(WCHUNK, k1 - o)
                            gt = g1p.tile([128, WCHUNK, D], TAB8, tag="g1")
                            nc.gpsimd.dma_gather(
                                gt[:, 0:m, :], Xs_d[j][:],
                                idx_t[:, o * 8:(o + m) * 8],
                                num_idxs=m * 128, num_idxs_reg=m * 128,
                                elem_size=D)
                            tiles.append(gt)
                        gt_tiles[(G, j)] = tiles

                    # consume block-major: self term, then both slices' chunks
                    for b in range(G * GS, (G + 1) * GS):
                        accA = app1.tile([128, 128], dt.float32,
                                         name=f"accA{b}", tag="accA")
                        accB = app1.tile([128, 128], dt.float32,
                                         name=f"accB{b}", tag="accB")
                        acc = [accA, accB]
                        nchunks = int(m_cnt[b, 0] + m_cnt[b, 1])
                        # self-loop: acc[k,n] += Xown[n,k] via identity rhs
                        for kc in range(KD):
                            nc.tensor.matmul(acc[kc][:], xo_t[:, b, kc, :],
                                             id_t[:], start=True,
                                             stop=(nchunks == 0))
                        done = 0
                        for j in range(SLICES):
                            for kk in range(int(off[b, j]),
                                            int(off[b, j] + m_cnt[b, j])):
                                gt, s = win_tile_slot(G, j, kk, gt_tiles)
                                ohap = make_oh(kk)
                                done += 1
                                last = (done == nchunks)
                                for kc in range(KD):
                                    nc.tensor.matmul(
                                        acc[kc][:],
                                        gt[:, s, kc * 128:(kc + 1) * 128],
                                        ohap, start=False, stop=last)

                        # evac cascade: AGG -> W1+b1 -> relu -> W2 -> *dinv^2
                        agg = evp.tile([128, KD, 128], dt.bfloat16, tag="agg")
                        nc.scalar.activation(agg[:, 0, :], accA[:],
                                             mybir.ActivationFunctionType.Copy)
                        nc.scalar.activation(agg[:, 1, :], accB[:],
                                             mybir.ActivationFunctionType.Copy)
                        h2 = hpp.tile([F1, 128], dt.float32, tag="h2")
                        for kc in range(KD):
                            nc.tensor.matmul(h2[:], W1_t[:, kc, :], agg[:, kc, :],
                                             start=(kc == 0), stop=False)
                        nc.tensor.matmul(h2[:], b1_t[:],
                                         recip_t[0:1, b * 128:(b + 1) * 128],
                                         start=False, stop=True)
                        h2s = evp.tile([F1, 128], dt.bfloat16, tag="h2s")
                        nc.scalar.activation(h2s[:], h2[:],
                                             mybir.ActivationFunctionType.Relu)
                        h3 = tpp.tile([128, F2], dt.float32, tag="h3")
                        nc.tensor.matmul(h3[:], h2s[:], W2_t[:],
                                         start=True, stop=True)
                        h3s = h3p.tile([128, F2], TAB8,
                                       name=f"h3s{b}", tag=f"h3s{b}")
                        nc.scalar.activation(h3s[:], h3[:],
                                             mybir.ActivationFunctionType.Copy,
                                             bias=0.0, scale=dinv2_t[:, b:b + 1])
                        h3s_tiles[b] = h3s
                        j_b = blk_slice(b)
                        r0 = b * 128 - BOUNDS[j_b]
                        w = nc.sync.dma_start(H3shs[j_b][r0:r0 + 128, :], h3s[:])
                        h3_writes[j_b].append(w)
                    del gt_tiles[(G, 0)], gt_tiles[(G, 1)]

                    # fire slice AllGather + expansion as soon as ready
                    for j in range(SLICES):
                        if (G + 1) * GS * 128 == BOUNDS[j + 1]:
                            cc = nc.gpsimd.collective_compute(
                                "AllGather", mybir.AluOpType.bypass,
                                replica_groups=[list(range(NCORES))],
                                ins=[H3shs[j][:]], outs=[H3tabs[j][:]])
                            for w in h3_writes[j]:
                                add_dep_helper(cc.ins, w.ins,
                                               reason="allgather reads H3 slice")
                            ex = nc.sync.dma_start(H3exp[j][:, 0:F2], H3tabs[j][:])
                            add_dep_helper(ex.ins, cc.ins,
                                           reason="expand reads allgathered tab")
                            exps[j] = ex

            # ================= layer 2 =================
            with tc.tile_pool(name="l2ps", bufs=3, space="PSUM") as app2:
                parks = {}
                for j in range(SLICES):
                    for G in range(NG):
                        k0, k1 = runs[(G, j)]
                        tiles = []
                        for o in range(k0, k1, WCHUNK):
                            m = min(WCHUNK, k1 - o)
                            gt8 = g2p.tile([128, WCHUNK, 256], TAB8, tag="g2")
                            gi = nc.gpsimd.dma_gather(
                                gt8[:, 0:m, :], H3exp[j][:],
                                idx_t[:, o * 8:(o + m) * 8],
                                num_idxs=m * 128, num_idxs_reg=m * 128,
                                elem_size=256, elem_step=256)
                            add_dep_helper(gi.ins, exps[j].ins,
                                           reason="gather reads expanded tab")
                            tiles.append(gt8)
                        gtt = {(G, j): tiles}

                        for b in range(G * GS, (G + 1) * GS):
                            m_j = int(m_cnt[b, j])
                            if j == 0:
                                a = app2.tile([128, F2], dt.float32,
                                              name=f"acc2_{b}_0", tag="acc2")
                                # self-loop: acc2[n,f] += h3s[n,f]
                                nc.tensor.matmul(a[:], id_t[:],
                                                 h3s_tiles[b][:],
                                                 start=True, stop=(m_j == 0))
                                for i, kk in enumerate(
                                        range(int(off[b, 0]),
                                              int(off[b, 0]) + m_j)):
                                    gt8, s = win_tile_slot(G, 0, kk, gtt)
                                    nc.tensor.matmul(
                                        a[:], make_oh(kk), gt8[:, s, 0:F2],
                                        start=False, stop=(i == m_j - 1))
                                pk = pkp.tile([128, F2], dt.float32,
                                              name=f"park{b}", tag=f"pk{b}")
                                nc.vector.tensor_copy(pk[:], a[:])
                                parks[b] = pk
                            else:
                                a = app2.tile([128, F2], dt.float32,
                                              name=f"acc2_{b}_1", tag="acc2")
                                for i, kk in enumerate(
                                        range(int(off[b, 1]),
                                              int(off[b, 1]) + m_j)):
                                    gt8, s = win_tile_slot(G, 1, kk, gtt)
                                    nc.tensor.matmul(
                                        a[:], make_oh(kk), gt8[:, s, 0:F2],
                                        start=(i == 0), stop=False)
                                # bias (b2 (x) sqrt(deg)) closes the group
                                nc.tensor.matmul(
                                    a[:], recip_t[0:1, b * 128:(b + 1) * 128],
                                    b2_t[:], start=(m_j == 0), stop=True)
                                tmp = evp.tile([128, F2], dt.float32, tag="tmp")
                                nc.vector.tensor_tensor(
                                    tmp[:], a[:], parks[b][:],
                                    mybir.AluOpType.add)
                                ost = evp.tile([128, F2], dt.float32, tag="ost")
                                nc.scalar.activation(
                                    ost[:], tmp[:],
                                    mybir.ActivationFunctionType.Copy,
                                    bias=0.0, scale=dinv1_t[:, b:b + 1])
                                nc.sync.dma_start(
                                    out_d[b * 128:(b + 1) * 128, :], ost[:])

    if not nc.is_finalized():
        nc.finalize()
    hoist_excess_waits(nc)
    return nc


# ---------------------------------------------------------------------------
def _kernel_impl(x, edge_index, W1, b1, W2, b2, ncores=NCORES):
    x = np.asarray(x, dtype=np.float32)
    edge_index = np.asarray(edge_index)
    W1 = np.asarray(W1, dtype=np.float32)
    b1 = np.asarray(b1, dtype=np.float32)
    W2 = np.asarray(W2, dtype=np.float32)
    b2 = np.asarray(b2, dtype=np.float32)
    N, D = x.shape
    F1 = W1.shape[1]
    F2 = W2.shape[1]

    cfg = _prepare(x, edge_index, ncores)
    nc = _build(cfg, F1, F2)

    bf16 = _np_dt(dt.bfloat16)
    in_maps = []
    for c in range(ncores):
        m = {f"Xs{j}": cfg['Xs'][j] for j in range(SLICES)}
        m.update({
            "Xown": cfg['Xown_np'][c],
            "W1": W1.astype(bf16),
            "W2": W2.astype(bf16),
            "b1": b1.reshape(1, F1).astype(bf16),
            "b2": b2.reshape(1, F2).astype(bf16),
            "iota64": cfg['iota64'],
            "id128": cfg['id128'],
            "idx": cfg['idx_np'][c],
            "dpair": cfg['dpair_np'][c],
            "dpar": cfg['dpar_np'][c],
            "dinv2": cfg['dinv2_np'][c],
            "dinv1": cfg['dinv1_np'][c],
            "recip": cfg['recip_np'][c].astype(bf16),
        })
        in_maps.append(m)
    res = run_bass_kernel_spmd(nc, in_maps, list(range(ncores)))

    NSH = cfg['NSH']
    out = np.empty((N, F2), np.float32)
    for c in range(ncores):
        o = res.results[c]["out"]            # [NSHP, F2]
        n0 = c * NSH
        n1 = min(N, n0 + NSH)
        out[n0:n1] = o[:n1 - n0]
    return out, res, nc, cfg


def kernel(x, edge_index, W1, b1, W2, b2):
    out, _, _, _ = _kernel_impl(x, edge_index, W1, b1, W2, b2)
    return out


# revision 9
# speedup vs baseline: 1.5260x; 1.0619x over previous
"""Two-layer GCN (GCNConv x2 + ReLU) on 8 Trainium2 NeuronCores.

Strategy (aggregate-first, dinv-folded, fp8 tables):
  - Nodes sharded by destination across 8 cores. Layer 1 aggregates RAW
    input features: each core gathers X'[src] rows per edge (X' = dinv*X,
    host-prescaled, fp8, laid out in 2 slice tables), scatter-adds them into
    per-dst-block PSUM accumulators (feature-major) via pure 0/1 one-hot
    matmuls, then applies W1 + bias + ReLU + W2 per 128-node block on-chip.
    No X@W1 table phase at all.
  - dinv normalization folded exactly: one-hots are pure 0/1; source dinv
    lives in the tables; dst dinv is applied as a per-partition activation
    scale (relu is positively homogeneous): H3' table gets dinv^2, final
    output gets dinv. Biases enter as K=1 matmuls of b (x) sqrt(deg).
  - H3' = dinv^2*relu(AGG@W1 + b1*sqrt(deg))@W2 stored fp8 e4m3 at 64
    features -> the AllGather moves 3.2MB total instead of 12.8MB. After the
    AllGather each slice is expanded to 256B-row stride (dma_gather needs
    elem/stride multiples of 256B); layer 2 gathers those rows node-major.
  - One-hots are built once per 128-edge chunk as packed uint16 pairs on DVE
    (2-byte dtype keeps the 4x DVE mode, 77ns) and bitcast to fp8 [128,128].
  - Self-loops are excluded from the edge lists (that removes exactly one
    full chunk per (block, slice-of-own-rows)): their contribution enters
    via identity matmuls against directly-read own-shard X' rows (layer 1)
    and the layer-1 H3 evac tiles still in SBUF (layer 2).
  - Edge chunks grouped (supergroup of 7 dst blocks) x (source slice of 2)
    so one SWDGE dma_gather covers up to 4096 edges (scratch ring enlarged),
    amortizing the ~1.1us per-gather prep cost. Chunks are CONSUMED
    block-major so only ~2 blocks' PSUM accumulators are live.
"""
import sys
sys.path.insert(0, '/opt/trn_rl_repo')
import numpy as np
import concourse.bass as bass
import concourse.bacc as bacc
import concourse.mybir as mybir
import bass_rust
from concourse.tile import TileContext
from concourse.tile_rust import add_dep_helper
from concourse.bass_utils import run_bass_kernel_spmd

dt = mybir.dt

NCORES = 8
SLICES = 2
WCHUNK = 16           # chunks per gather window
SCRATCH = 32768        # SWDGE ring: 2048 descriptors
TAB8 = dt.float8e4     # table dtype (e4m3)
PAD_DST = 999.0        # one-hot miss value for padding edges


def _np_dt(d):
    return mybir.dt.np(d)


# ---------------------------------------------------------------------------
# walrus in this toolchain rejects >1 attached sem wait on several opcodes;
# hoist extras into standalone InstEventSemaphore instructions just before.
def hoist_excess_waits(nc, max_attached=1):
    n_new = 0
    for f in nc.m.functions:
        for bb in f.blocks:
            insts = bb.instructions  # live list
            i = 0
            while i < len(insts):
                inst = insts[i]
                si = inst.sync_info
                if si is not None and inst.engine is not None:
                    waits = list(si.on_wait)
                    imm = [w for w in waits if w.wait_reg is None]
                    other = [w for w in waits if w.wait_reg is not None]
                    budget = max_attached - len(other)
                    if len(imm) > budget:
                        if budget > 0:
                            extra, keep = imm[:-budget], imm[-budget:]
                        else:
                            extra, keep = imm, []
                        for w in extra:
                            ev = mybir.InstEventSemaphore(
                                name=f"I-hoistw{n_new}", ins=[], outs=[])
                            ev.engine = inst.engine
                            h = bass_rust.SemaphoreHandle(name=w.ant_name, num=w.id)
                            bass_rust.wait_op(ev, h, w.wait_value, "sem-ge", True)
                            insts.insert(i, ev)
                            i += 1
                            n_new += 1
                        si.on_wait = other + keep
                i += 1
    return n_new


# ---------------------------------------------------------------------------
# host-side graph preprocessing
def _prepare(x, edge_index, ncores):
    x = np.asarray(x, dtype=np.float32)
    N, D = x.shape
    NSH = (N + ncores - 1) // ncores            # nodes per shard (6250)
    TS = (NSH + 127) // 128                     # dst blocks per shard (49)
    GS = max(s for s in range(1, 9) if TS % s == 0)   # blocks per supergroup
    NG = TS // GS                               # supergroups per shard
    NSHP = TS * 128                             # padded shard rows

    # slice bounds in shard rows (multiples of GS*128); near-even split
    gb, rem = NG // SLICES, NG % SLICES
    parts = [gb + (1 if i < rem else 0) for i in range(SLICES)]
    BOUNDS = [0]
    for p in parts:
        BOUNDS.append(BOUNDS[-1] + p * GS * 128)
    RSLS = [BOUNDS[i + 1] - BOUNDS[i] for i in range(SLICES)]
    assert all(ncores * r < 32768 for r in RSLS), RSLS
    BARR = np.array(BOUNDS)
    RARR = np.array(RSLS)

    src = edge_index[0].astype(np.int64)        # self-loops handled separately
    dst = edge_index[1].astype(np.int64)
    E = len(src)

    deg = np.bincount(dst, minlength=N).astype(np.float32) + 1.0  # + self loop
    dinv = 1.0 / np.sqrt(deg)

    # source table row (slice-local): node s -> shard c, local l;
    # slice j of l; row = c*RSL_j + (l - B_j)
    src_c, src_l = src // NSH, src % NSH
    src_j = (np.searchsorted(BARR, src_l, side='right') - 1).astype(np.int64)
    src_row = src_c * RARR[src_j] + (src_l - BARR[src_j])

    dst_c, dst_l = dst // NSH, dst % NSH
    dst_b = dst_l // 128
    dst_p = dst_l % 128

    # cell = (core, block, slice); shared chunk schedule = max count per cell
    cell = (dst_c * TS + dst_b) * SLICES + src_j
    counts = np.bincount(cell, minlength=ncores * TS * SLICES)
    counts3 = counts.reshape(ncores, TS, SLICES)
    m_cnt = (counts3.max(axis=0) + 127) // 128          # [TS, SLICES]

    # chunk offsets in gather order: for G: for j: for b in G
    off = np.zeros((TS, SLICES), np.int64)
    chunk_block = []
    runs = {}            # (G, j) -> (k_start, k_end)
    k = 0
    for G in range(NG):
        for j in range(SLICES):
            k0 = k
            for b in range(G * GS, (G + 1) * GS):
                off[b, j] = k
                m = int(m_cnt[b, j])
                chunk_block.extend([b] * m)
                k += m
            runs[(G, j)] = (k0, k)
    NCHT = k
    chunk_block = np.array(chunk_block, np.int64)

    # rank of each edge within its cell (stable order)
    order = np.argsort(cell, kind='stable')
    starts = np.zeros(ncores * TS * SLICES + 1, np.int64)
    starts[1:] = np.cumsum(counts)
    rank = np.empty(E, np.int64)
    rank[order] = np.arange(E) - starts[cell[order]]

    slot = off[dst_b, src_j] * 128 + rank       # flat slot per edge

    idx_np = np.zeros((ncores, 128, NCHT * 8), np.int16)
    dstl_np = np.full((ncores, 128, NCHT), PAD_DST, np.float32)
    dpair_np = np.full((ncores, 128, NCHT), PAD_DST, np.float32)
    dpar_np = np.zeros((ncores, 128, NCHT), np.float32)
    for c in range(ncores):
        m = dst_c == c
        fsrc = np.zeros(NCHT * 128, np.int64)
        fdst = np.full(NCHT * 128, PAD_DST, np.float32)
        fsrc[slot[m]] = src_row[m]
        fdst[slot[m]] = dst_p[m]
        i16 = fsrc.astype(np.int16).reshape(-1, 16).T      # [16, NCHT*8]
        idx_np[c] = np.tile(i16, (8, 1))
        dstl_np[c] = fdst.reshape(NCHT, 128).T
        pair = np.where(fdst == PAD_DST, PAD_DST, np.floor(fdst / 2.0))
        par = np.where(fdst % 2.0 == 0.0, 56.0, 14336.0)   # fp8 1.0 lo/hi byte
        dpair_np[c] = pair.reshape(NCHT, 128).T
        dpar_np[c] = par.reshape(NCHT, 128).T

    # X' tables per slice (bf16), row = c*RSL_j + (l - B_j); pad rows zero
    f8 = _np_dt(TAB8)
    bf16 = _np_dt(dt.bfloat16)
    xp = (x * dinv[:, None]).astype(np.float32)
    Xs = []
    for j in range(SLICES):
        t = np.zeros((ncores * RSLS[j], D), np.float32)
        for c in range(ncores):
            l0, l1 = BOUNDS[j], BOUNDS[j + 1]
            n0 = c * NSH + l0
            n1 = min(c * NSH + min(l1, NSH), N)
            if n1 > n0:
                t[c * RSLS[j]:c * RSLS[j] + (n1 - n0)] = xp[n0:n1]
        Xs.append(t.astype(bf16))

    # per-core own-shard X' rows (plain order) for the self-loop term
    Xown_np = np.zeros((ncores, NSHP, D), np.float32)
    for c in range(ncores):
        n0, n1 = c * NSH, min((c + 1) * NSH, N)
        Xown_np[c, :n1 - n0] = xp[n0:n1]
    Xown_np = Xown_np.astype(f8)

    # per-core dst-side scales
    dinv2_np = np.ones((ncores, 128, TS), np.float32)
    dinv1_np = np.ones((ncores, 128, TS), np.float32)
    recip_np = np.zeros((ncores, 1, NSHP), np.float32)
    for c in range(ncores):
        n0, n1 = c * NSH, min((c + 1) * NSH, N)
        dloc = np.ones(NSHP, np.float32)
        dloc[:n1 - n0] = dinv[n0:n1]
        dinv2_np[c] = (dloc ** 2).reshape(TS, 128).T
        dinv1_np[c] = dloc.reshape(TS, 128).T
        r = np.zeros(NSHP, np.float32)
        r[:n1 - n0] = 1.0 / dinv[n0:n1]
        recip_np[c, 0] = r

    iota64 = np.tile(np.arange(64, dtype=np.uint16)[None, :], (128, 1)).copy()
    iota = np.tile(np.arange(128, dtype=np.float32)[None, :], (128, 1)).copy()
    id128 = np.eye(128, dtype=np.float32)

    return dict(N=N, D=D, NSH=NSH, TS=TS, GS=GS, NG=NG, NSHP=NSHP,
                BOUNDS=BOUNDS, RSLS=RSLS, NCHT=NCHT, runs=runs,
                m_cnt=m_cnt, off=off, chunk_block=chunk_block,
                idx_np=idx_np, dstl_np=dstl_np, dpair_np=dpair_np,
                dpar_np=dpar_np, iota=iota,
                Xs=Xs, Xown_np=Xown_np, dinv2_np=dinv2_np,
                dinv1_np=dinv1_np, recip_np=recip_np,
                iota64=iota64, id128=id128)


# ---------------------------------------------------------------------------
def _build(cfg, F1, F2):
    D, TS, GS, NG = cfg['D'], cfg['TS'], cfg['GS'], cfg['NG']
    NSHP, NCHT = cfg['NSHP'], cfg['NCHT']
    BOUNDS, RSLS = cfg['BOUNDS'], cfg['RSLS']
    runs, m_cnt, off = cfg['runs'], cfg['m_cnt'], cfg['off']
    KD = D // 128

    nc = bacc.Bacc(None, target_bir_lowering=False,
                   dynamic_dma_scratch_size=SCRATCH)
    Xs_d = [nc.declare_dram_parameter(f"Xs{j}", [NCORES * RSLS[j], D],
                                      dt.bfloat16, isOutput=False)
            for j in range(SLICES)]
    Xown_d = nc.declare_dram_parameter("Xown", [NSHP, D], TAB8, isOutput=False)
    W1_d = nc.declare_dram_parameter("W1", [D, F1], dt.bfloat16, isOutput=False)
    W2_d = nc.declare_dram_parameter("W2", [F1, F2], dt.bfloat16, isOutput=False)
    b1_d = nc.declare_dram_parameter("b1", [1, F1], dt.bfloat16, isOutput=False)
    b2_d = nc.declare_dram_parameter("b2", [1, F2], dt.bfloat16, isOutput=False)
    iota64_d = nc.declare_dram_parameter("iota64", [128, 64], dt.uint16, isOutput=False)
    iota_d = nc.declare_dram_parameter("iota", [128, 128], dt.bfloat16, isOutput=False)
    id128_d = nc.declare_dram_parameter("id128", [128, 128], TAB8, isOutput=False)
    dstl_d = nc.declare_dram_parameter("dstl", [128, NCHT], dt.float32, isOutput=False)
    idx_d = nc.declare_dram_parameter("idx", [128, NCHT * 8], dt.int16, isOutput=False)
    dpair_d = nc.declare_dram_parameter("dpair", [128, NCHT], dt.float32, isOutput=False)
    dpar_d = nc.declare_dram_parameter("dpar", [128, NCHT], dt.float32, isOutput=False)
    dinv2_d = nc.declare_dram_parameter("dinv2", [128, TS], dt.float32, isOutput=False)
    dinv1_d = nc.declare_dram_parameter("dinv1", [128, TS], dt.float32, isOutput=False)
    recip_d = nc.declare_dram_parameter("recip", [1, NSHP], dt.bfloat16, isOutput=False)
    out_d = nc.declare_dram_parameter("out", [NSHP, F2], dt.float32, isOutput=True)

    H3shs = [nc.dram_tensor(f"H3sh{j}", [RSLS[j], F2], TAB8)
             for j in range(SLICES)]
    H3tabs = [nc.dram_tensor(f"H3tab{j}", [NCORES * RSLS[j], F2], TAB8,
                             addr_space="Shared") for j in range(SLICES)]
    H3exp = [nc.dram_tensor(f"H3exp{j}", [NCORES * RSLS[j], 256], TAB8)
             for j in range(SLICES)]

    def blk_slice(b):
        return 0 if (b + 1) * 128 <= BOUNDS[1] else 1

    max_run = max(k1 - k0 for (k1, k0) in ((b, a) for (a, b) in runs.values()))
    wpr = -(-max_run // WCHUNK)          # windows per run

    with TileContext(nc) as tc:
        with (
            tc.tile_pool(name="const", bufs=1) as cp,
            tc.tile_pool(name="l1gt", bufs=2 * wpr + 2) as g1p,
            tc.tile_pool(name="l2gt", bufs=wpr + 2) as g2p,
            tc.tile_pool(name="oh16", bufs=10) as ohp,
            tc.tile_pool(name="evac", bufs=3) as evp,
            tc.tile_pool(name="h3sb", bufs=1) as h3p,
            tc.tile_pool(name="park", bufs=1) as pkp,
        ):
            # ---- constants / metadata resident in SBUF ----
            iota64_t = cp.tile([128, 64], dt.uint16, tag="iota64")
            nc.sync.dma_start(iota64_t[:], iota64_d[:])
            id_t = cp.tile([128, 128], TAB8, tag="id128")
            nc.sync.dma_start(id_t[:], id128_d[:])
            iota_t = cp.tile([128, 128], dt.bfloat16, tag="iota")
            nc.sync.dma_start(iota_t[:], iota_d[:])
            dstl_t = cp.tile([128, NCHT], dt.float32, tag="dstl")
            nc.sync.dma_start(dstl_t[:], dstl_d[:])
            W1_t = cp.tile([128, KD, F1], dt.bfloat16, tag="W1")
            nc.sync.dma_start(W1_t[:], W1_d[:].rearrange("(k p) f -> p k f", p=128))
            W2_t = cp.tile([F1, F2], dt.bfloat16, tag="W2")
            nc.sync.dma_start(W2_t[:], W2_d[:])
            b1_t = cp.tile([1, F1], dt.bfloat16, tag="b1")
            nc.sync.dma_start(b1_t[:], b1_d[:])
            b2_t = cp.tile([1, F2], dt.bfloat16, tag="b2")
            nc.sync.dma_start(b2_t[:], b2_d[:])
            xo_t = cp.tile([128, TS, KD, 128], TAB8, tag="Xown")
            nc.sync.dma_start(
                xo_t[:], Xown_d[:].rearrange("(t p) (k f) -> p t k f",
                                             p=128, k=KD))
            idx_t = cp.tile([128, NCHT * 8], dt.int16, tag="idx")
            nc.sync.dma_start(idx_t[:], idx_d[:])
            dpair_t = cp.tile([128, NCHT], dt.float32, tag="dpair")
            nc.sync.dma_start(dpair_t[:], dpair_d[:])
            dpar_t = cp.tile([128, NCHT], dt.float32, tag="dpar")
            nc.sync.dma_start(dpar_t[:], dpar_d[:])
            dinv2_t = cp.tile([128, TS], dt.float32, tag="dinv2")
            nc.sync.dma_start(dinv2_t[:], dinv2_d[:])
            dinv1_t = cp.tile([128, TS], dt.float32, tag="dinv1")
            nc.sync.dma_start(dinv1_t[:], dinv1_d[:])
            recip_t = cp.tile([1, NSHP], dt.bfloat16, tag="recip")
            nc.sync.dma_start(recip_t[:], recip_d[:])

            def make_oh1(kk):
                oh = ohp.tile([128, 128], dt.bfloat16, tag="ohb")
                nc.vector.tensor_scalar(
                    oh[:], iota_t[:], dstl_t[:, kk:kk + 1], None,
                    mybir.AluOpType.is_equal)
                return oh[:]

            def make_oh(kk):
                oh = ohp.tile([128, 64], dt.uint16, tag="oh16")
                nc.vector.tensor_scalar(
                    oh[:], iota64_t[:], dpair_t[:, kk:kk + 1],
                    dpar_t[:, kk:kk + 1],
                    mybir.AluOpType.is_equal, mybir.AluOpType.mult)
                return oh[:].bitcast(TAB8)

            def win_tile_slot(G, j, kk, tiles):
                """gather tile + slot for chunk kk of run (G, j)."""
                k0, _ = runs[(G, j)]
                w = (kk - k0) // WCHUNK
                return tiles[(G, j)][w], (kk - k0) % WCHUNK

            h3_writes = {j: [] for j in range(SLICES)}
            h3s_tiles = {}
            exps = {}

            # ================= layer 1 =================
            with (
                tc.tile_pool(name="l1ps", bufs=2, space="PSUM") as app1,
                tc.tile_pool(name="h2ps", bufs=1, space="PSUM") as hpp,
                tc.tile_pool(name="h3ps", bufs=1, space="PSUM") as tpp,
            ):
                gt_tiles = {}
                for G in range(NG):
                    # issue all gathers of this supergroup (both slices)
                    for j in range(SLICES):
                        k0, k1 = runs[(G, j)]
                        tiles = []
                        for o in range(k0, k1, WCHUNK):
                            m = min(WCHUNK, k1 - o)
                            gt = g1p.tile([128, WCHUNK, D], dt.bfloat16, tag="g1")
                            nc.gpsimd.dma_gather(
                                gt[:, 0:m, :], Xs_d[j][:],
                                idx_t[:, o * 8:(o + m) * 8],
                                num_idxs=m * 128, num_idxs_reg=m * 128,
                                elem_size=D)
                            tiles.append(gt)
                        gt_tiles[(G, j)] = tiles

                    # consume block-major: self term, then both slices' chunks
                    for b in range(G * GS, (G + 1) * GS):
                        accA = app1.tile([128, 128], dt.float32,
                                         name=f"accA{b}", tag="accA")
                        accB = app1.tile([128, 128], dt.float32,
                                         name=f"accB{b}", tag="accB")
                        acc = [accA, accB]
                        nchunks = int(m_cnt[b, 0] + m_cnt[b, 1])
                        # self-loop: acc[k,n] += Xown[n,k] via identity rhs
                        for kc in range(KD):
                            nc.tensor.matmul(acc[kc][:], xo_t[:, b, kc, :],
                                             id_t[:], start=True,
                                             stop=(nchunks == 0))
                        done = 0
                        for j in range(SLICES):
                            for kk in range(int(off[b, j]),
                                            int(off[b, j] + m_cnt[b, j])):
                                gt, s = win_tile_slot(G, j, kk, gt_tiles)
                                ohap = make_oh1(kk)
                                done += 1
                                last = (done == nchunks)
                                for kc in range(KD):
                                    nc.tensor.matmul(
                                        acc[kc][:],
                                        gt[:, s, kc * 128:(kc + 1) * 128],
                                        ohap, start=False, stop=last)

                        # evac cascade: AGG -> W1+b1 -> relu -> W2 -> *dinv^2
                        agg = evp.tile([128, KD, 128], dt.bfloat16, tag="agg")
                        nc.scalar.activation(agg[:, 0, :], accA[:],
                                             mybir.ActivationFunctionType.Copy)
                        nc.scalar.activation(agg[:, 1, :], accB[:],
                                             mybir.ActivationFunctionType.Copy)
                        h2 = hpp.tile([F1, 128], dt.float32, tag="h2")
                        for kc in range(KD):
                            nc.tensor.matmul(h2[:], W1_t[:, kc, :], agg[:, kc, :],
                                             start=(kc == 0), stop=False)
                        nc.tensor.matmul(h2[:], b1_t[:],
                                         recip_t[0:1, b * 128:(b + 1) * 128],
                                         start=False, stop=True)
                        h2s = evp.tile([F1, 128], dt.bfloat16, tag="h2s")
                        nc.scalar.activation(h2s[:], h2[:],
                                             mybir.ActivationFunctionType.Relu)
                        h3 = tpp.tile([128, F2], dt.float32, tag="h3")
                        nc.tensor.matmul(h3[:], h2s[:], W2_t[:],
                                         start=True, stop=True)
                        h3s = h3p.tile([128, F2], TAB8,
                                       name=f"h3s{b}", tag=f"h3s{b}")
                        nc.scalar.activation(h3s[:], h3[:],
                                             mybir.ActivationFunctionType.Copy,
                                             bias=0.0, scale=dinv2_t[:, b:b + 1])
                        h3s_tiles[b] = h3s
                        j_b = blk_slice(b)
                        r0 = b * 128 - BOUNDS[j_b]
                        w = nc.sync.dma_start(H3shs[j_b][r0:r0 + 128, :], h3s[:])
                        h3_writes[j_b].append(w)
                    del gt_tiles[(G, 0)], gt_tiles[(G, 1)]

                    # fire slice AllGather + expansion as soon as ready
                    for j in range(SLICES):
                        if (G + 1) * GS * 128 == BOUNDS[j + 1]:
                            cc = nc.gpsimd.collective_compute(
                                "AllGather", mybir.AluOpType.bypass,
                                replica_groups=[list(range(NCORES))],
                                ins=[H3shs[j][:]], outs=[H3tabs[j][:]])
                            for w in h3_writes[j]:
                                add_dep_helper(cc.ins, w.ins,
                                               reason="allgather reads H3 slice")
                            ex = nc.sync.dma_start(H3exp[j][:, 0:F2], H3tabs[j][:])
                            add_dep_helper(ex.ins, cc.ins,
                                           reason="expand reads allgathered tab")
                            exps[j] = ex

            # ================= layer 2 =================
            with tc.tile_pool(name="l2ps", bufs=3, space="PSUM") as app2:
                parks = {}
                for j in range(SLICES):
                    for G in range(NG):
                        k0, k1 = runs[(G, j)]
                        tiles = []
                        for o in range(k0, k1, WCHUNK):
                            m = min(WCHUNK, k1 - o)
                            gt8 = g2p.tile([128, WCHUNK, 256], TAB8, tag="g2")
                            gi = nc.gpsimd.dma_gather(
                                gt8[:, 0:m, :], H3exp[j][:],
                                idx_t[:, o * 8:(o + m) * 8],
                                num_idxs=m * 128, num_idxs_reg=m * 128,
                                elem_size=256, elem_step=256)
                            add_dep_helper(gi.ins, exps[j].ins,
                                           reason="gather reads expanded tab")
                            tiles.append(gt8)
                        gtt = {(G, j): tiles}

                        for b in range(G * GS, (G + 1) * GS):
                            m_j = int(m_cnt[b, j])
                            if j == 0:
                                a = app2.tile([128, F2], dt.float32,
                                              name=f"acc2_{b}_0", tag="acc2")
                                # self-loop: acc2[n,f] += h3s[n,f]
                                nc.tensor.matmul(a[:], id_t[:],
                                                 h3s_tiles[b][:],
                                                 start=True, stop=(m_j == 0))
                                for i, kk in enumerate(
                                        range(int(off[b, 0]),
                                              int(off[b, 0]) + m_j)):
                                    gt8, s = win_tile_slot(G, 0, kk, gtt)
                                    nc.tensor.matmul(
                                        a[:], make_oh(kk), gt8[:, s, 0:F2],
                                        start=False, stop=(i == m_j - 1))
                                pk = pkp.tile([128, F2], dt.bfloat16,
                                              name=f"park{b}", tag=f"pk{b}")
                                nc.vector.tensor_copy(pk[:], a[:])
                                parks[b] = pk
                            else:
                                a = app2.tile([128, F2], dt.float32,
                                              name=f"acc2_{b}_1", tag="acc2")
                                for i, kk in enumerate(
                                        range(int(off[b, 1]),
                                              int(off[b, 1]) + m_j)):
                                    gt8, s = win_tile_slot(G, 1, kk, gtt)
                                    nc.tensor.matmul(
                                        a[:], make_oh(kk), gt8[:, s, 0:F2],
                                        start=(i == 0), stop=False)
                                # bias (b2 (x) sqrt(deg)) closes the group
                                nc.tensor.matmul(
                                    a[:], recip_t[0:1, b * 128:(b + 1) * 128],
                                    b2_t[:], start=(m_j == 0), stop=True)
                                tmp = evp.tile([128, F2], dt.float32, tag="tmp")
                                nc.vector.tensor_tensor(
                                    tmp[:], a[:], parks[b][:],
                                    mybir.AluOpType.add)
                                ost = evp.tile([128, F2], dt.float32, tag="ost")
                                nc.scalar.activation(
                                    ost[:], tmp[:],
                                    mybir.ActivationFunctionType.Copy,
                                    bias=0.0, scale=dinv1_t[:, b:b + 1])
                                nc.sync.dma_start(
                                    out_d[b * 128:(b + 1) * 128, :], ost[:])

    if not nc.is_finalized():
        nc.finalize()
    hoist_excess_waits(nc)
    return nc


# ---------------------------------------------------------------------------
def _kernel_impl(x, edge_index, W1, b1, W2, b2, ncores=NCORES):
    x = np.asarray(x, dtype=np.float32)
    edge_index = np.asarray(edge_index)
    W1 = np.asarray(W1, dtype=np.float32)
    b1 = np.asarray(b1, dtype=np.float32)
    W2 = np.asarray(W2, dtype=np.float32)
    b2 = np.asarray(b2, dtype=np.float32)
    N, D = x.shape
    F1 = W1.shape[1]
    F2 = W2.shape[1]

    cfg = _prepare(x, edge_index, ncores)
    nc = _build(cfg, F1, F2)

    bf16 = _np_dt(dt.bfloat16)
    in_maps = []
    for c in range(ncores):
        m = {f"Xs{j}": cfg['Xs'][j] for j in range(SLICES)}
        m.update({
            "Xown": cfg['Xown_np'][c],
            "W1": W1.astype(bf16),
            "W2": W2.astype(bf16),
            "b1": b1.reshape(1, F1).astype(bf16),
            "b2": b2.reshape(1, F2).astype(bf16),
            "iota64": cfg['iota64'],
            "iota": cfg['iota'].astype(bf16),
            "id128": cfg['id128'].astype(_np_dt(dt.float8e4)),
            "dstl": cfg['dstl_np'][c],
            "idx": cfg['idx_np'][c],
            "dpair": cfg['dpair_np'][c],
            "dpar": cfg['dpar_np'][c],
            "dinv2": cfg['dinv2_np'][c],
            "dinv1": cfg['dinv1_np'][c],
            "recip": cfg['recip_np'][c].astype(bf16),
        })
        in_maps.append(m)
    res = run_bass_kernel_spmd(nc, in_maps, list(range(ncores)))

    NSH = cfg['NSH']
    out = np.empty((N, F2), np.float32)
    for c in range(ncores):
        o = res.results[c]["out"]            # [NSHP, F2]
        n0 = c * NSH
        n1 = min(N, n0 + NSH)
        out[n0:n1] = o[:n1 - n0]
    return out, res, nc, cfg


def kernel(x, edge_index, W1, b1, W2, b2):
    out, _, _, _ = _kernel_impl(x, edge_index, W1, b1, W2, b2)
    return out


# revision 10
# speedup vs baseline: 1.5623x; 1.0238x over previous
"""Two-layer GCN (GCNConv x2 + ReLU) on 8 Trainium2 NeuronCores.

Strategy (aggregate-first, dinv-folded, fp8 tables):
  - Nodes sharded by destination across 8 cores. Layer 1 aggregates RAW
    input features: each core gathers X'[src] rows per edge (X' = dinv*X,
    host-prescaled, fp8, laid out in 2 slice tables), scatter-adds them into
    per-dst-block PSUM accumulators (feature-major) via pure 0/1 one-hot
    matmuls, then applies W1 + bias + ReLU + W2 per 128-node block on-chip.
    No X@W1 table phase at all.
  - dinv normalization folded exactly: one-hots are pure 0/1; source dinv
    lives in the tables; dst dinv is applied as a per-partition activation
    scale (relu is positively homogeneous): H3' table gets dinv^2, final
    output gets dinv. Biases enter as K=1 matmuls of b (x) sqrt(deg).
  - H3' = dinv^2*relu(AGG@W1 + b1*sqrt(deg))@W2 stored fp8 e4m3 at 64
    features -> the AllGather moves 3.2MB total instead of 12.8MB. After the
    AllGather each slice is expanded to 256B-row stride (dma_gather needs
    elem/stride multiples of 256B); layer 2 gathers those rows node-major.
  - One-hots are built once per 128-edge chunk as packed uint16 pairs on DVE
    (2-byte dtype keeps the 4x DVE mode, 77ns) and bitcast to fp8 [128,128].
  - Self-loops are excluded from the edge lists (that removes exactly one
    full chunk per (block, slice-of-own-rows)): their contribution enters
    via identity matmuls against directly-read own-shard X' rows (layer 1)
    and the layer-1 H3 evac tiles still in SBUF (layer 2).
  - Edge chunks grouped (supergroup of 7 dst blocks) x (source slice of 2)
    so one SWDGE dma_gather covers up to 4096 edges (scratch ring enlarged),
    amortizing the ~1.1us per-gather prep cost. Chunks are CONSUMED
    block-major so only ~2 blocks' PSUM accumulators are live.
"""
import sys
sys.path.insert(0, '/opt/trn_rl_repo')
import numpy as np
import concourse.bass as bass
import concourse.bacc as bacc
import concourse.mybir as mybir
import bass_rust
from concourse.tile import TileContext
from concourse.tile_rust import add_dep_helper
from concourse.bass_utils import run_bass_kernel_spmd

dt = mybir.dt

NCORES = 8
SLICES = 2
WCHUNK = 8            # chunks per gather window (1024 idx = HW SWDGE ring cap)
SCRATCH = 16384        # SWDGE ring: 1024 descriptors (HW cap)
TAB8 = dt.float8e4     # table dtype (e4m3)
PAD_DST = 999.0        # one-hot miss value for padding edges


def _np_dt(d):
    return mybir.dt.np(d)


# ---------------------------------------------------------------------------
# walrus in this toolchain rejects >1 attached sem wait on several opcodes;
# hoist extras into standalone InstEventSemaphore instructions just before.
def hoist_excess_waits(nc, max_attached=1):
    n_new = 0
    for f in nc.m.functions:
        for bb in f.blocks:
            insts = bb.instructions  # live list
            i = 0
            while i < len(insts):
                inst = insts[i]
                si = inst.sync_info
                if si is not None and inst.engine is not None:
                    waits = list(si.on_wait)
                    imm = [w for w in waits if w.wait_reg is None]
                    other = [w for w in waits if w.wait_reg is not None]
                    budget = max_attached - len(other)
                    if len(imm) > budget:
                        if budget > 0:
                            extra, keep = imm[:-budget], imm[-budget:]
                        else:
                            extra, keep = imm, []
                        for w in extra:
                            ev = mybir.InstEventSemaphore(
                                name=f"I-hoistw{n_new}", ins=[], outs=[])
                            ev.engine = inst.engine
                            h = bass_rust.SemaphoreHandle(name=w.ant_name, num=w.id)
                            bass_rust.wait_op(ev, h, w.wait_value, "sem-ge", True)
                            insts.insert(i, ev)
                            i += 1
                            n_new += 1
                        si.on_wait = other + keep
                i += 1
    return n_new


# ---------------------------------------------------------------------------
# host-side graph preprocessing
def _prepare(x, edge_index, ncores):
    x = np.asarray(x, dtype=np.float32)
    N, D = x.shape
    NSH = (N + ncores - 1) // ncores            # nodes per shard (6250)
    TS = (NSH + 127) // 128                     # dst blocks per shard (49)
    GS = max(s for s in range(1, 9) if TS % s == 0)   # blocks per supergroup
    NG = TS // GS                               # supergroups per shard
    NSHP = TS * 128                             # padded shard rows

    # slice bounds in shard rows (multiples of GS*128); near-even split
    gb, rem = NG // SLICES, NG % SLICES
    parts = [gb + (1 if i < rem else 0) for i in range(SLICES)]
    BOUNDS = [0]
    for p in parts:
        BOUNDS.append(BOUNDS[-1] + p * GS * 128)
    RSLS = [BOUNDS[i + 1] - BOUNDS[i] for i in range(SLICES)]
    assert all(ncores * r < 32768 for r in RSLS), RSLS
    BARR = np.array(BOUNDS)
    RARR = np.array(RSLS)

    src = edge_index[0].astype(np.int64)        # self-loops handled separately
    dst = edge_index[1].astype(np.int64)
    E = len(src)

    deg = np.bincount(dst, minlength=N).astype(np.float32) + 1.0  # + self loop
    dinv = 1.0 / np.sqrt(deg)

    # source table row (slice-local): node s -> shard c, local l;
    # slice j of l; row = c*RSL_j + (l - B_j)
    src_c, src_l = src // NSH, src % NSH
    src_j = (np.searchsorted(BARR, src_l, side='right') - 1).astype(np.int64)
    src_row = src_c * RARR[src_j] + (src_l - BARR[src_j])

    dst_c, dst_l = dst // NSH, dst % NSH
    dst_b = dst_l // 128
    dst_p = dst_l % 128

    # cell = (core, block, slice); shared chunk schedule = max count per cell
    cell = (dst_c * TS + dst_b) * SLICES + src_j
    counts = np.bincount(cell, minlength=ncores * TS * SLICES)
    counts3 = counts.reshape(ncores, TS, SLICES)
    m_cnt = (counts3.max(axis=0) + 127) // 128          # [TS, SLICES]

    # chunk offsets in gather order: for G: for j: for b in G
    off = np.zeros((TS, SLICES), np.int64)
    chunk_block = []
    runs = {}            # (G, j) -> (k_start, k_end)
    k = 0
    for G in range(NG):
        for j in range(SLICES):
            k0 = k
            for b in range(G * GS, (G + 1) * GS):
                off[b, j] = k
                m = int(m_cnt[b, j])
                chunk_block.extend([b] * m)
                k += m
            runs[(G, j)] = (k0, k)
    NCHT = k
    chunk_block = np.array(chunk_block, np.int64)

    # rank of each edge within its cell (stable order)
    order = np.argsort(cell, kind='stable')
    starts = np.zeros(ncores * TS * SLICES + 1, np.int64)
    starts[1:] = np.cumsum(counts)
    rank = np.empty(E, np.int64)
    rank[order] = np.arange(E) - starts[cell[order]]

    slot = off[dst_b, src_j] * 128 + rank       # flat slot per edge

    idx_np = np.zeros((ncores, 128, NCHT * 8), np.int16)
    dstl_np = np.full((ncores, 128, NCHT), PAD_DST, np.float32)
    dpair_np = np.full((ncores, 128, NCHT), PAD_DST, np.float32)
    dpar_np = np.zeros((ncores, 128, NCHT), np.float32)
    for c in range(ncores):
        m = dst_c == c
        fsrc = np.zeros(NCHT * 128, np.int64)
        fdst = np.full(NCHT * 128, PAD_DST, np.float32)
        fsrc[slot[m]] = src_row[m]
        fdst[slot[m]] = dst_p[m]
        i16 = fsrc.astype(np.int16).reshape(-1, 16).T      # [16, NCHT*8]
        idx_np[c] = np.tile(i16, (8, 1))
        dstl_np[c] = fdst.reshape(NCHT, 128).T
        pair = np.where(fdst == PAD_DST, PAD_DST, np.floor(fdst / 2.0))
        par = np.where(fdst % 2.0 == 0.0, 56.0, 14336.0)   # fp8 1.0 lo/hi byte
        dpair_np[c] = pair.reshape(NCHT, 128).T
        dpar_np[c] = par.reshape(NCHT, 128).T

    # X' tables per slice (bf16), row = c*RSL_j + (l - B_j); pad rows zero
    f8 = _np_dt(TAB8)
    bf16 = _np_dt(dt.bfloat16)
    xp = (x * dinv[:, None]).astype(np.float32)
    Xs = []
    for j in range(SLICES):
        t = np.zeros((ncores * RSLS[j], D), np.float32)
        for c in range(ncores):
            l0, l1 = BOUNDS[j], BOUNDS[j + 1]
            n0 = c * NSH + l0
            n1 = min(c * NSH + min(l1, NSH), N)
            if n1 > n0:
                t[c * RSLS[j]:c * RSLS[j] + (n1 - n0)] = xp[n0:n1]
        Xs.append(t.astype(bf16))

    # per-core own-shard X' rows (plain order) for the self-loop term
    Xown_np = np.zeros((ncores, NSHP, D), np.float32)
    for c in range(ncores):
        n0, n1 = c * NSH, min((c + 1) * NSH, N)
        Xown_np[c, :n1 - n0] = xp[n0:n1]
    Xown_np = Xown_np.astype(f8)

    # per-core dst-side scales
    dinv2_np = np.ones((ncores, 128, TS), np.float32)
    dinv1_np = np.ones((ncores, 128, TS), np.float32)
    recip_np = np.zeros((ncores, 1, NSHP), np.float32)
    for c in range(ncores):
        n0, n1 = c * NSH, min((c + 1) * NSH, N)
        dloc = np.ones(NSHP, np.float32)
        dloc[:n1 - n0] = dinv[n0:n1]
        dinv2_np[c] = (dloc ** 2).reshape(TS, 128).T
        dinv1_np[c] = dloc.reshape(TS, 128).T
        r = np.zeros(NSHP, np.float32)
        r[:n1 - n0] = 1.0 / dinv[n0:n1]
        recip_np[c, 0] = r

    iota64 = np.tile(np.arange(64, dtype=np.uint16)[None, :], (128, 1)).copy()
    iota = np.tile(np.arange(128, dtype=np.float32)[None, :], (128, 1)).copy()
    id128 = np.eye(128, dtype=np.float32)

    return dict(N=N, D=D, NSH=NSH, TS=TS, GS=GS, NG=NG, NSHP=NSHP,
                BOUNDS=BOUNDS, RSLS=RSLS, NCHT=NCHT, runs=runs,
                m_cnt=m_cnt, off=off, chunk_block=chunk_block,
                idx_np=idx_np, dstl_np=dstl_np, dpair_np=dpair_np,
                dpar_np=dpar_np, iota=iota,
                Xs=Xs, Xown_np=Xown_np, dinv2_np=dinv2_np,
                dinv1_np=dinv1_np, recip_np=recip_np,
                iota64=iota64, id128=id128)


# ---------------------------------------------------------------------------
def _build(cfg, F1, F2):
    D, TS, GS, NG = cfg['D'], cfg['TS'], cfg['GS'], cfg['NG']
    NSHP, NCHT = cfg['NSHP'], cfg['NCHT']
    BOUNDS, RSLS = cfg['BOUNDS'], cfg['RSLS']
    runs, m_cnt, off = cfg['runs'], cfg['m_cnt'], cfg['off']
    KD = D // 128

    nc = bacc.Bacc(None, target_bir_lowering=False,
                   dynamic_dma_scratch_size=SCRATCH)
    Xs_d = [nc.declare_dram_parameter(f"Xs{j}", [NCORES * RSLS[j], D],
                                      dt.bfloat16, isOutput=False)
            for j in range(SLICES)]
    Xown_d = nc.declare_dram_parameter("Xown", [NSHP, D], TAB8, isOutput=False)
    W1_d = nc.declare_dram_parameter("W1", [D, F1], dt.bfloat16, isOutput=False)
    W2_d = nc.declare_dram_parameter("W2", [F1, F2], dt.bfloat16, isOutput=False)
    b1_d = nc.declare_dram_parameter("b1", [1, F1], dt.bfloat16, isOutput=False)
    b2_d = nc.declare_dram_parameter("b2", [1, F2], dt.bfloat16, isOutput=False)
    iota64_d = nc.declare_dram_parameter("iota64", [128, 64], dt.uint16, isOutput=False)
    iota_d = nc.declare_dram_parameter("iota", [128, 128], dt.bfloat16, isOutput=False)
    id128_d = nc.declare_dram_parameter("id128", [128, 128], TAB8, isOutput=False)
    dstl_d = nc.declare_dram_parameter("dstl", [128, NCHT], dt.float32, isOutput=False)
    idx_d = nc.declare_dram_parameter("idx", [128, NCHT * 8], dt.int16, isOutput=False)
    dpair_d = nc.declare_dram_parameter("dpair", [128, NCHT], dt.float32, isOutput=False)
    dpar_d = nc.declare_dram_parameter("dpar", [128, NCHT], dt.float32, isOutput=False)
    dinv2_d = nc.declare_dram_parameter("dinv2", [128, TS], dt.float32, isOutput=False)
    dinv1_d = nc.declare_dram_parameter("dinv1", [128, TS], dt.float32, isOutput=False)
    recip_d = nc.declare_dram_parameter("recip", [1, NSHP], dt.bfloat16, isOutput=False)
    out_d = nc.declare_dram_parameter("out", [NSHP, F2], dt.float32, isOutput=True)

    H3shs = [nc.dram_tensor(f"H3sh{j}", [RSLS[j], F2], TAB8)
             for j in range(SLICES)]
    H3tabs = [nc.dram_tensor(f"H3tab{j}", [NCORES * RSLS[j], F2], TAB8,
                             addr_space="Shared") for j in range(SLICES)]
    H3exp = [nc.dram_tensor(f"H3exp{j}", [NCORES * RSLS[j], 256], TAB8)
             for j in range(SLICES)]

    def blk_slice(b):
        return 0 if (b + 1) * 128 <= BOUNDS[1] else 1

    max_run = max(k1 - k0 for (k1, k0) in ((b, a) for (a, b) in runs.values()))
    wpr = -(-max_run // WCHUNK)          # windows per run

    with TileContext(nc) as tc:
        with (
            tc.tile_pool(name="const", bufs=1) as cp,
            tc.tile_pool(name="l1gt", bufs=2 * wpr + 2) as g1p,
            tc.tile_pool(name="l2gt", bufs=wpr + 2) as g2p,
            tc.tile_pool(name="oh16", bufs=10) as ohp,
            tc.tile_pool(name="evac", bufs=3) as evp,
            tc.tile_pool(name="h3sb", bufs=1) as h3p,
            tc.tile_pool(name="park", bufs=1) as pkp,
        ):
            # ---- constants / metadata resident in SBUF ----
            iota64_t = cp.tile([128, 64], dt.uint16, tag="iota64")
            nc.sync.dma_start(iota64_t[:], iota64_d[:])
            id_t = cp.tile([128, 128], TAB8, tag="id128")
            nc.sync.dma_start(id_t[:], id128_d[:])
            iota_t = cp.tile([128, 128], dt.bfloat16, tag="iota")
            nc.sync.dma_start(iota_t[:], iota_d[:])
            dstl_t = cp.tile([128, NCHT], dt.float32, tag="dstl")
            nc.sync.dma_start(dstl_t[:], dstl_d[:])
            W1_t = cp.tile([128, KD, F1], dt.bfloat16, tag="W1")
            nc.sync.dma_start(W1_t[:], W1_d[:].rearrange("(k p) f -> p k f", p=128))
            W2_t = cp.tile([F1, F2], dt.bfloat16, tag="W2")
            nc.sync.dma_start(W2_t[:], W2_d[:])
            b1_t = cp.tile([1, F1], dt.bfloat16, tag="b1")
            nc.sync.dma_start(b1_t[:], b1_d[:])
            b2_t = cp.tile([1, F2], dt.bfloat16, tag="b2")
            nc.sync.dma_start(b2_t[:], b2_d[:])
            xo_t = cp.tile([128, TS, KD, 128], TAB8, tag="Xown")
            nc.sync.dma_start(
                xo_t[:], Xown_d[:].rearrange("(t p) (k f) -> p t k f",
                                             p=128, k=KD))
            idx_t = cp.tile([128, NCHT * 8], dt.int16, tag="idx")
            nc.sync.dma_start(idx_t[:], idx_d[:])
            dpair_t = cp.tile([128, NCHT], dt.float32, tag="dpair")
            nc.sync.dma_start(dpair_t[:], dpair_d[:])
            dpar_t = cp.tile([128, NCHT], dt.float32, tag="dpar")
            nc.sync.dma_start(dpar_t[:], dpar_d[:])
            dinv2_t = cp.tile([128, TS], dt.float32, tag="dinv2")
            nc.sync.dma_start(dinv2_t[:], dinv2_d[:])
            dinv1_t = cp.tile([128, TS], dt.float32, tag="dinv1")
            nc.sync.dma_start(dinv1_t[:], dinv1_d[:])
            recip_t = cp.tile([1, NSHP], dt.bfloat16, tag="recip")
            nc.sync.dma_start(recip_t[:], recip_d[:])

            def make_oh1(kk):
                oh = ohp.tile([128, 128], dt.bfloat16, tag="ohb")
                nc.vector.tensor_scalar(
                    oh[:], iota_t[:], dstl_t[:, kk:kk + 1], None,
                    mybir.AluOpType.is_equal)
                return oh[:]

            def make_oh(kk):
                oh = ohp.tile([128, 64], dt.uint16, tag="oh16")
                nc.vector.tensor_scalar(
                    oh[:], iota64_t[:], dpair_t[:, kk:kk + 1],
                    dpar_t[:, kk:kk + 1],
                    mybir.AluOpType.is_equal, mybir.AluOpType.mult)
                return oh[:].bitcast(TAB8)

            def win_tile_slot(G, j, kk, tiles):
                """gather tile + slot for chunk kk of run (G, j)."""
                k0, _ = runs[(G, j)]
                w = (kk - k0) // WCHUNK
                return tiles[(G, j)][w], (kk - k0) % WCHUNK

            h3_writes = {j: [] for j in range(SLICES)}
            h3s_tiles = {}
            exps = {}

            # ================= layer 1 =================
            with (
                tc.tile_pool(name="l1ps", bufs=2, space="PSUM") as app1,
                tc.tile_pool(name="h2ps", bufs=1, space="PSUM") as hpp,
                tc.tile_pool(name="h3ps", bufs=1, space="PSUM") as tpp,
            ):
                gt_tiles = {}
                for G in range(NG):
                    # issue all gathers of this supergroup (both slices)
                    for j in range(SLICES):
                        k0, k1 = runs[(G, j)]
                        tiles = []
                        for o in range(k0, k1, WCHUNK):
                            m = min(WCHUNK, k1 - o)
                            gt = g1p.tile([128, WCHUNK, D], dt.bfloat16, tag="g1")
                            nc.gpsimd.dma_gather(
                                gt[:, 0:m, :], Xs_d[j][:],
                                idx_t[:, o * 8:(o + m) * 8],
                                num_idxs=m * 128, num_idxs_reg=m * 128,
                                elem_size=D)
                            tiles.append(gt)
                        gt_tiles[(G, j)] = tiles

                    # consume block-major: self term, then both slices' chunks
                    for b in range(G * GS, (G + 1) * GS):
                        accA = app1.tile([128, 128], dt.float32,
                                         name=f"accA{b}", tag="accA")
                        accB = app1.tile([128, 128], dt.float32,
                                         name=f"accB{b}", tag="accB")
                        acc = [accA, accB]
                        nchunks = int(m_cnt[b, 0] + m_cnt[b, 1])
                        # self-loop: acc[k,n] += Xown[n,k] via identity rhs
                        for kc in range(KD):
                            nc.tensor.matmul(acc[kc][:], xo_t[:, b, kc, :],
                                             id_t[:], start=True,
                                             stop=(nchunks == 0))
                        done = 0
                        for j in range(SLICES):
                            for kk in range(int(off[b, j]),
                                            int(off[b, j] + m_cnt[b, j])):
                                gt, s = win_tile_slot(G, j, kk, gt_tiles)
                                ohap = make_oh1(kk)
                                done += 1
                                last = (done == nchunks)
                                for kc in range(KD):
                                    nc.tensor.matmul(
                                        acc[kc][:],
                                        gt[:, s, kc * 128:(kc + 1) * 128],
                                        ohap, start=False, stop=last)

                        # evac cascade: AGG -> W1+b1 -> relu -> W2 -> *dinv^2
                        agg = evp.tile([128, KD, 128], dt.bfloat16, tag="agg")
                        nc.scalar.activation(agg[:, 0, :], accA[:],
                                             mybir.ActivationFunctionType.Copy)
                        nc.scalar.activation(agg[:, 1, :], accB[:],
                                             mybir.ActivationFunctionType.Copy)
                        h2 = hpp.tile([F1, 128], dt.float32, tag="h2")
                        for kc in range(KD):
                            nc.tensor.matmul(h2[:], W1_t[:, kc, :], agg[:, kc, :],
                                             start=(kc == 0), stop=False)
                        nc.tensor.matmul(h2[:], b1_t[:],
                                         recip_t[0:1, b * 128:(b + 1) * 128],
                                         start=False, stop=True)
                        h2s = evp.tile([F1, 128], dt.bfloat16, tag="h2s")
                        nc.scalar.activation(h2s[:], h2[:],
                                             mybir.ActivationFunctionType.Relu)
                        h3 = tpp.tile([128, F2], dt.float32, tag="h3")
                        nc.tensor.matmul(h3[:], h2s[:], W2_t[:],
                                         start=True, stop=True)
                        h3s = h3p.tile([128, F2], TAB8,
                                       name=f"h3s{b}", tag=f"h3s{b}")
                        nc.scalar.activation(h3s[:], h3[:],
                                             mybir.ActivationFunctionType.Copy,
                                             bias=0.0, scale=dinv2_t[:, b:b + 1])
                        h3s_tiles[b] = h3s
                        j_b = blk_slice(b)
                        r0 = b * 128 - BOUNDS[j_b]
                        w = nc.sync.dma_start(H3shs[j_b][r0:r0 + 128, :], h3s[:])
                        h3_writes[j_b].append(w)
                    del gt_tiles[(G, 0)], gt_tiles[(G, 1)]

                    # fire slice AllGather + expansion as soon as ready
                    for j in range(SLICES):
                        if (G + 1) * GS * 128 == BOUNDS[j + 1]:
                            cc = nc.gpsimd.collective_compute(
                                "AllGather", mybir.AluOpType.bypass,
                                replica_groups=[list(range(NCORES))],
                                ins=[H3shs[j][:]], outs=[H3tabs[j][:]])
                            for w in h3_writes[j]:
                                add_dep_helper(cc.ins, w.ins,
                                               reason="allgather reads H3 slice")
                            ex = nc.sync.dma_start(H3exp[j][:, 0:F2], H3tabs[j][:])
                            add_dep_helper(ex.ins, cc.ins,
                                           reason="expand reads allgathered tab")
                            exps[j] = ex

            # ================= layer 2 =================
            with tc.tile_pool(name="l2ps", bufs=3, space="PSUM") as app2:
                parks = {}
                for j in range(SLICES):
                    for G in range(NG):
                        k0, k1 = runs[(G, j)]
                        tiles = []
                        for o in range(k0, k1, WCHUNK):
                            m = min(WCHUNK, k1 - o)
                            gt8 = g2p.tile([128, WCHUNK, 256], TAB8, tag="g2")
                            gi = nc.gpsimd.dma_gather(
                                gt8[:, 0:m, :], H3exp[j][:],
                                idx_t[:, o * 8:(o + m) * 8],
                                num_idxs=m * 128, num_idxs_reg=m * 128,
                                elem_size=256, elem_step=256)
                            add_dep_helper(gi.ins, exps[j].ins,
                                           reason="gather reads expanded tab")
                            tiles.append(gt8)
                        gtt = {(G, j): tiles}

                        for b in range(G * GS, (G + 1) * GS):
                            m_j = int(m_cnt[b, j])
                            if j == 0:
                                a = app2.tile([128, F2], dt.float32,
                                              name=f"acc2_{b}_0", tag="acc2")
                                # self-loop: acc2[n,f] += h3s[n,f]
                                nc.tensor.matmul(a[:], id_t[:],
                                                 h3s_tiles[b][:],
                                                 start=True, stop=(m_j == 0))
                                for i, kk in enumerate(
                                        range(int(off[b, 0]),
                                              int(off[b, 0]) + m_j)):
                                    gt8, s = win_tile_slot(G, 0, kk, gtt)
                                    nc.tensor.matmul(
                                        a[:], make_oh(kk), gt8[:, s, 0:F2],
                                        start=False, stop=(i == m_j - 1))
                                pk = pkp.tile([128, F2], dt.bfloat16,
                                              name=f"park{b}", tag=f"pk{b}")
                                nc.vector.tensor_copy(pk[:], a[:])
                                parks[b] = pk
                            else:
                                a = app2.tile([128, F2], dt.float32,
                                              name=f"acc2_{b}_1", tag="acc2")
                                for i, kk in enumerate(
                                        range(int(off[b, 1]),
                                              int(off[b, 1]) + m_j)):
                                    gt8, s = win_tile_slot(G, 1, kk, gtt)
                                    nc.tensor.matmul(
                                        a[:], make_oh(kk), gt8[:, s, 0:F2],
                                        start=(i == 0), stop=False)
                                # bias (b2 (x) sqrt(deg)) closes the group
                                nc.tensor.matmul(
                                    a[:], recip_t[0:1, b * 128:(b + 1) * 128],
                                    b2_t[:], start=(m_j == 0), stop=True)
                                tmp = evp.tile([128, F2], dt.float32, tag="tmp")
                                nc.vector.tensor_tensor(
                                    tmp[:], a[:], parks[b][:],
                                    mybir.AluOpType.add)
                                ost = evp.tile([128, F2], dt.float32, tag="ost")
                                nc.scalar.activation(
                                    ost[:], tmp[:],
                                    mybir.ActivationFunctionType.Copy,
                                    bias=0.0, scale=dinv1_t[:, b:b + 1])
                                nc.sync.dma_start(
                                    out_d[b * 128:(b + 1) * 128, :], ost[:])

    if not nc.is_finalized():
        nc.finalize()
    hoist_excess_waits(nc)
    return nc


# ---------------------------------------------------------------------------
def _kernel_impl(x, edge_index, W1, b1, W2, b2, ncores=NCORES):
    x = np.asarray(x, dtype=np.float32)
    edge_index = np.asarray(edge_index)
    W1 = np.asarray(W1, dtype=np.float32)
    b1 = np.asarray(b1, dtype=np.float32)
    W2 = np.asarray(W2, dtype=np.float32)
    b2 = np.asarray(b2, dtype=np.float32)
    N, D = x.shape
    F1 = W1.shape[1]
    F2 = W2.shape[1]

    cfg = _prepare(x, edge_index, ncores)
    nc = _build(cfg, F1, F2)

    bf16 = _np_dt(dt.bfloat16)
    in_maps = []
    for c in range(ncores):
        m = {f"Xs{j}": cfg['Xs'][j] for j in range(SLICES)}
        m.update({
            "Xown": cfg['Xown_np'][c],
            "W1": W1.astype(bf16),
            "W2": W2.astype(bf16),
            "b1": b1.reshape(1, F1).astype(bf16),
            "b2": b2.reshape(1, F2).astype(bf16),
            "iota64": cfg['iota64'],
            "iota": cfg['iota'].astype(bf16),
            "id128": cfg['id128'].astype(_np_dt(dt.float8e4)),
            "dstl": cfg['dstl_np'][c],
            "idx": cfg['idx_np'][c],
            "dpair": cfg['dpair_np'][c],
            "dpar": cfg['dpar_np'][c],
            "dinv2": cfg['dinv2_np'][c],
            "dinv1": cfg['dinv1_np'][c],
            "recip": cfg['recip_np'][c].astype(bf16),
        })
        in_maps.append(m)
    res = run_bass_kernel_spmd(nc, in_maps, list(range(ncores)))

    NSH = cfg['NSH']
    out = np.empty((N, F2), np.float32)
    for c in range(ncores):
        o = res.results[c]["out"]            # [NSHP, F2]
        n0 = c * NSH
        n1 = min(N, n0 + NSH)
        out[n0:n1] = o[:n1 - n0]
    return out, res, nc, cfg


def kernel(x, edge_index, W1, b1, W2, b2):
    out, _, _, _ = _kernel_impl(x, edge_index, W1, b1, W2, b2)
    return out


# revision 11
# speedup vs baseline: 1.6027x; 1.0258x over previous
"""Two-layer GCN (GCNConv x2 + ReLU) on 8 Trainium2 NeuronCores.

Strategy (aggregate-first, dinv-folded, fp8 tables):
  - Nodes sharded by destination across 8 cores. Layer 1 aggregates RAW
    input features: each core gathers X'[src] rows per edge (X' = dinv*X,
    host-prescaled, fp8, laid out in 2 slice tables), scatter-adds them into
    per-dst-block PSUM accumulators (feature-major) via pure 0/1 one-hot
    matmuls, then applies W1 + bias + ReLU + W2 per 128-node block on-chip.
    No X@W1 table phase at all.
  - dinv normalization folded exactly: one-hots are pure 0/1; source dinv
    lives in the tables; dst dinv is applied as a per-partition activation
    scale (relu is positively homogeneous): H3' table gets dinv^2, final
    output gets dinv. Biases enter as K=1 matmuls of b (x) sqrt(deg).
  - H3' = dinv^2*relu(AGG@W1 + b1*sqrt(deg))@W2 stored fp8 e4m3 at 64
    features -> the AllGather moves 3.2MB total instead of 12.8MB. After the
    AllGather each slice is expanded to 256B-row stride (dma_gather needs
    elem/stride multiples of 256B); layer 2 gathers those rows node-major.
  - One-hots are built once per 128-edge chunk as packed uint16 pairs on DVE
    (2-byte dtype keeps the 4x DVE mode, 77ns) and bitcast to fp8 [128,128].
  - Self-loops are excluded from the edge lists (that removes exactly one
    full chunk per (block, slice-of-own-rows)): their contribution enters
    via identity matmuls against directly-read own-shard X' rows (layer 1)
    and the layer-1 H3 evac tiles still in SBUF (layer 2).
  - Edge chunks grouped (supergroup of 7 dst blocks) x (source slice of 2)
    so one SWDGE dma_gather covers up to 4096 edges (scratch ring enlarged),
    amortizing the ~1.1us per-gather prep cost. Chunks are CONSUMED
    block-major so only ~2 blocks' PSUM accumulators are live.
"""
import sys
sys.path.insert(0, '/opt/trn_rl_repo')
import numpy as np
import concourse.bass as bass
import concourse.bacc as bacc
import concourse.mybir as mybir
import bass_rust
from concourse.tile import TileContext
from concourse.tile_rust import add_dep_helper
from concourse.bass_utils import run_bass_kernel_spmd

dt = mybir.dt

NCORES = 8
SLICES = 2
WCHUNK = 8            # chunks per gather window (1024 idx = HW SWDGE ring cap)
SCRATCH = 16384        # SWDGE ring: 1024 descriptors (HW cap)
TAB8 = dt.float8e4     # table dtype (e4m3)
PAD_DST = 999.0        # one-hot miss value for padding edges


def _np_dt(d):
    return mybir.dt.np(d)


# ---------------------------------------------------------------------------
# walrus in this toolchain rejects >1 attached sem wait on several opcodes;
# hoist extras into standalone InstEventSemaphore instructions just before.
def hoist_excess_waits(nc, max_attached=1):
    n_new = 0
    for f in nc.m.functions:
        for bb in f.blocks:
            insts = bb.instructions  # live list
            i = 0
            while i < len(insts):
                inst = insts[i]
                si = inst.sync_info
                if si is not None and inst.engine is not None:
                    waits = list(si.on_wait)
                    imm = [w for w in waits if w.wait_reg is None]
                    other = [w for w in waits if w.wait_reg is not None]
                    budget = max_attached - len(other)
                    if len(imm) > budget:
                        if budget > 0:
                            extra, keep = imm[:-budget], imm[-budget:]
                        else:
                            extra, keep = imm, []
                        for w in extra:
                            ev = mybir.InstEventSemaphore(
                                name=f"I-hoistw{n_new}", ins=[], outs=[])
                            ev.engine = inst.engine
                            h = bass_rust.SemaphoreHandle(name=w.ant_name, num=w.id)
                            bass_rust.wait_op(ev, h, w.wait_value, "sem-ge", True)
                            insts.insert(i, ev)
                            i += 1
                            n_new += 1
                        si.on_wait = other + keep
                i += 1
    return n_new


# ---------------------------------------------------------------------------
# host-side graph preprocessing
def _prepare(x, edge_index, ncores):
    x = np.asarray(x, dtype=np.float32)
    N, D = x.shape
    NSH = (N + ncores - 1) // ncores            # nodes per shard (6250)
    TS = (NSH + 127) // 128                     # dst blocks per shard (49)
    GS = max(s for s in range(1, 9) if TS % s == 0)   # blocks per supergroup
    NG = TS // GS                               # supergroups per shard
    NSHP = TS * 128                             # padded shard rows

    # slice bounds in shard rows (multiples of GS*128); near-even split
    gb, rem = NG // SLICES, NG % SLICES
    parts = [gb + (1 if i < rem else 0) for i in range(SLICES)]
    BOUNDS = [0]
    for p in parts:
        BOUNDS.append(BOUNDS[-1] + p * GS * 128)
    RSLS = [BOUNDS[i + 1] - BOUNDS[i] for i in range(SLICES)]
    assert all(ncores * r < 32768 for r in RSLS), RSLS
    BARR = np.array(BOUNDS)
    RARR = np.array(RSLS)

    src = edge_index[0].astype(np.int64)        # self-loops handled separately
    dst = edge_index[1].astype(np.int64)
    E = len(src)

    deg = np.bincount(dst, minlength=N).astype(np.float32) + 1.0  # + self loop
    dinv = 1.0 / np.sqrt(deg)

    # source table row (slice-local): node s -> shard c, local l;
    # slice j of l; row = c*RSL_j + (l - B_j)
    src_c, src_l = src // NSH, src % NSH
    src_j = (np.searchsorted(BARR, src_l, side='right') - 1).astype(np.int64)
    src_row = src_c * RARR[src_j] + (src_l - BARR[src_j])

    dst_c, dst_l = dst // NSH, dst % NSH
    dst_b = dst_l // 128
    dst_p = dst_l % 128

    # cell = (core, block, slice); shared schedule sized by max count per cell
    cell = (dst_c * TS + dst_b) * SLICES + src_j
    counts = np.bincount(cell, minlength=ncores * TS * SLICES)
    counts3 = counts.reshape(ncores, TS, SLICES)
    mx_cnt = counts3.max(axis=0)                        # [TS, SLICES]

    # continuous packing: per run (G, j), cells' edge spans (length mx_cnt)
    # are laid back-to-back; chunks of 128 descs may straddle cells. Each
    # (cell, chunk) intersection is a SEGMENT with its own one-hot column.
    cell_off = np.zeros((TS, SLICES), np.int64)   # desc offset of cell
    runs = {}                                     # (G, j) -> (k0, k1) chunks
    seg_block = []                                # per segment: block
    segs_by_block = {b: [] for b in range(TS)}    # b -> [(j, kk, col)]
    first_seg_of_cell = np.zeros((TS, SLICES), np.int64)
    first_chunk_of_cell = np.zeros((TS, SLICES), np.int64)
    k = 0                                         # chunk counter
    for G in range(NG):
        for j in range(SLICES):
            k0 = k
            d = 0                                 # desc offset within run
            for b in range(G * GS, (G + 1) * GS):
                cnt = int(mx_cnt[b, j])
                cell_off[b, j] = k0 * 128 + d
                if cnt > 0:
                    first_seg_of_cell[b, j] = len(seg_block)
                    first_chunk_of_cell[b, j] = k0 + d // 128
                    for kk in range(k0 + d // 128,
                                    k0 + (d + cnt - 1) // 128 + 1):
                        col = len(seg_block)
                        seg_block.append(b)
                        segs_by_block[b].append((j, kk, col))
                d += cnt
            k = k0 + (d + 127) // 128
            runs[(G, j)] = (k0, k)
    NCHT = k
    NSEG = len(seg_block)

    # rank of each edge within its cell (stable order)
    order = np.argsort(cell, kind='stable')
    starts = np.zeros(ncores * TS * SLICES + 1, np.int64)
    starts[1:] = np.cumsum(counts)
    rank = np.empty(E, np.int64)
    rank[order] = np.arange(E) - starts[cell[order]]

    slot = cell_off[dst_b, src_j] + rank        # flat desc slot per edge
    edge_col = (first_seg_of_cell[dst_b, src_j]
                + slot // 128 - first_chunk_of_cell[dst_b, src_j])

    idx_np = np.zeros((ncores, 128, NCHT * 8), np.int16)
    dstl_np = np.full((ncores, 128, NSEG), PAD_DST, np.float32)
    dpair_np = np.full((ncores, 128, NSEG), PAD_DST, np.float32)
    dpar_np = np.zeros((ncores, 128, NSEG), np.float32)
    for c in range(ncores):
        m = dst_c == c
        fsrc = np.zeros(NCHT * 128, np.int64)
        fsrc[slot[m]] = src_row[m]
        i16 = fsrc.astype(np.int16).reshape(-1, 16).T      # [16, NCHT*8]
        idx_np[c] = np.tile(i16, (8, 1))
        p_m = slot[m] % 128
        col_m = edge_col[m]
        dstl_np[c][p_m, col_m] = dst_p[m]
        dpair_np[c][p_m, col_m] = np.floor(dst_p[m] / 2.0)
        dpar_np[c][p_m, col_m] = np.where(dst_p[m] % 2 == 0, 56.0, 14336.0)

    # X' tables per slice (bf16), row = c*RSL_j + (l - B_j); pad rows zero
    f8 = _np_dt(TAB8)
    bf16 = _np_dt(dt.bfloat16)
    xp = (x * dinv[:, None]).astype(np.float32)
    Xs = []
    for j in range(SLICES):
        t = np.zeros((ncores * RSLS[j], D), np.float32)
        for c in range(ncores):
            l0, l1 = BOUNDS[j], BOUNDS[j + 1]
            n0 = c * NSH + l0
            n1 = min(c * NSH + min(l1, NSH), N)
            if n1 > n0:
                t[c * RSLS[j]:c * RSLS[j] + (n1 - n0)] = xp[n0:n1]
        Xs.append(t.astype(bf16))

    # per-core own-shard X' rows (plain order) for the self-loop term
    Xown_np = np.zeros((ncores, NSHP, D), np.float32)
    for c in range(ncores):
        n0, n1 = c * NSH, min((c + 1) * NSH, N)
        Xown_np[c, :n1 - n0] = xp[n0:n1]
    Xown_np = Xown_np.astype(f8)

    # per-core dst-side scales
    dinv2_np = np.ones((ncores, 128, TS), np.float32)
    dinv1_np = np.ones((ncores, 128, TS), np.float32)
    recip_np = np.zeros((ncores, 1, NSHP), np.float32)
    for c in range(ncores):
        n0, n1 = c * NSH, min((c + 1) * NSH, N)
        dloc = np.ones(NSHP, np.float32)
        dloc[:n1 - n0] = dinv[n0:n1]
        dinv2_np[c] = (dloc ** 2).reshape(TS, 128).T
        dinv1_np[c] = dloc.reshape(TS, 128).T
        r = np.zeros(NSHP, np.float32)
        r[:n1 - n0] = 1.0 / dinv[n0:n1]
        recip_np[c, 0] = r

    iota64 = np.tile(np.arange(64, dtype=np.uint16)[None, :], (128, 1)).copy()
    iota = np.tile(np.arange(128, dtype=np.float32)[None, :], (128, 1)).copy()
    id128 = np.eye(128, dtype=np.float32)

    return dict(N=N, D=D, NSH=NSH, TS=TS, GS=GS, NG=NG, NSHP=NSHP,
                BOUNDS=BOUNDS, RSLS=RSLS, NCHT=NCHT, NSEG=NSEG, runs=runs,
                segs_by_block=segs_by_block,
                idx_np=idx_np, dstl_np=dstl_np, dpair_np=dpair_np,
                dpar_np=dpar_np, iota=iota,
                Xs=Xs, Xown_np=Xown_np, dinv2_np=dinv2_np,
                dinv1_np=dinv1_np, recip_np=recip_np,
                iota64=iota64, id128=id128)


# ---------------------------------------------------------------------------
def _build(cfg, F1, F2):
    D, TS, GS, NG = cfg['D'], cfg['TS'], cfg['GS'], cfg['NG']
    NSHP, NCHT, NSEG = cfg['NSHP'], cfg['NCHT'], cfg['NSEG']
    BOUNDS, RSLS = cfg['BOUNDS'], cfg['RSLS']
    runs = cfg['runs']
    segs_by_block = cfg['segs_by_block']
    KD = D // 128

    nc = bacc.Bacc(None, target_bir_lowering=False,
                   dynamic_dma_scratch_size=SCRATCH)
    Xs_d = [nc.declare_dram_parameter(f"Xs{j}", [NCORES * RSLS[j], D],
                                      dt.bfloat16, isOutput=False)
            for j in range(SLICES)]
    Xown_d = nc.declare_dram_parameter("Xown", [NSHP, D], TAB8, isOutput=False)
    W1_d = nc.declare_dram_parameter("W1", [D, F1], dt.bfloat16, isOutput=False)
    W2_d = nc.declare_dram_parameter("W2", [F1, F2], dt.bfloat16, isOutput=False)
    b1_d = nc.declare_dram_parameter("b1", [1, F1], dt.bfloat16, isOutput=False)
    b2_d = nc.declare_dram_parameter("b2", [1, F2], dt.bfloat16, isOutput=False)
    iota64_d = nc.declare_dram_parameter("iota64", [128, 64], dt.uint16, isOutput=False)
    iota_d = nc.declare_dram_parameter("iota", [128, 128], dt.bfloat16, isOutput=False)
    id128_d = nc.declare_dram_parameter("id128", [128, 128], TAB8, isOutput=False)
    dstl_d = nc.declare_dram_parameter("dstl", [128, NSEG], dt.float32, isOutput=False)
    idx_d = nc.declare_dram_parameter("idx", [128, NCHT * 8], dt.int16, isOutput=False)
    dpair_d = nc.declare_dram_parameter("dpair", [128, NSEG], dt.float32, isOutput=False)
    dpar_d = nc.declare_dram_parameter("dpar", [128, NSEG], dt.float32, isOutput=False)
    dinv2_d = nc.declare_dram_parameter("dinv2", [128, TS], dt.float32, isOutput=False)
    dinv1_d = nc.declare_dram_parameter("dinv1", [128, TS], dt.float32, isOutput=False)
    recip_d = nc.declare_dram_parameter("recip", [1, NSHP], dt.bfloat16, isOutput=False)
    out_d = nc.declare_dram_parameter("out", [NSHP, F2], dt.float32, isOutput=True)

    H3shs = [nc.dram_tensor(f"H3sh{j}", [RSLS[j], F2], TAB8)
             for j in range(SLICES)]
    H3tabs = [nc.dram_tensor(f"H3tab{j}", [NCORES * RSLS[j], F2], TAB8,
                             addr_space="Shared") for j in range(SLICES)]
    H3exp = [nc.dram_tensor(f"H3exp{j}", [NCORES * RSLS[j], 256], TAB8)
             for j in range(SLICES)]

    def blk_slice(b):
        return 0 if (b + 1) * 128 <= BOUNDS[1] else 1

    max_run = max(k1 - k0 for (k1, k0) in ((b, a) for (a, b) in runs.values()))
    wpr = -(-max_run // WCHUNK)          # windows per run

    with TileContext(nc) as tc:
        with (
            tc.tile_pool(name="const", bufs=1) as cp,
            tc.tile_pool(name="l1gt", bufs=2 * wpr + 2) as g1p,
            tc.tile_pool(name="l2gt", bufs=wpr + 2) as g2p,
            tc.tile_pool(name="oh16", bufs=10) as ohp,
            tc.tile_pool(name="evac", bufs=3) as evp,
            tc.tile_pool(name="h3sb", bufs=1) as h3p,
            tc.tile_pool(name="park", bufs=1) as pkp,
        ):
            # ---- constants / metadata resident in SBUF ----
            iota64_t = cp.tile([128, 64], dt.uint16, tag="iota64")
            nc.sync.dma_start(iota64_t[:], iota64_d[:])
            id_t = cp.tile([128, 128], TAB8, tag="id128")
            nc.sync.dma_start(id_t[:], id128_d[:])
            iota_t = cp.tile([128, 128], dt.bfloat16, tag="iota")
            nc.sync.dma_start(iota_t[:], iota_d[:])
            dstl_t = cp.tile([128, NSEG], dt.float32, tag="dstl")
            nc.sync.dma_start(dstl_t[:], dstl_d[:])
            W1_t = cp.tile([128, KD, F1], dt.bfloat16, tag="W1")
            nc.sync.dma_start(W1_t[:], W1_d[:].rearrange("(k p) f -> p k f", p=128))
            W2_t = cp.tile([F1, F2], dt.bfloat16, tag="W2")
            nc.sync.dma_start(W2_t[:], W2_d[:])
            b1_t = cp.tile([1, F1], dt.bfloat16, tag="b1")
            nc.sync.dma_start(b1_t[:], b1_d[:])
            b2_t = cp.tile([1, F2], dt.bfloat16, tag="b2")
            nc.sync.dma_start(b2_t[:], b2_d[:])
            xo_t = cp.tile([128, TS, KD, 128], TAB8, tag="Xown")
            nc.sync.dma_start(
                xo_t[:], Xown_d[:].rearrange("(t p) (k f) -> p t k f",
                                             p=128, k=KD))
            idx_t = cp.tile([128, NCHT * 8], dt.int16, tag="idx")
            nc.sync.dma_start(idx_t[:], idx_d[:])
            dpair_t = cp.tile([128, NSEG], dt.float32, tag="dpair")
            nc.sync.dma_start(dpair_t[:], dpair_d[:])
            dpar_t = cp.tile([128, NSEG], dt.float32, tag="dpar")
            nc.sync.dma_start(dpar_t[:], dpar_d[:])
            dinv2_t = cp.tile([128, TS], dt.float32, tag="dinv2")
            nc.sync.dma_start(dinv2_t[:], dinv2_d[:])
            dinv1_t = cp.tile([128, TS], dt.float32, tag="dinv1")
            nc.sync.dma_start(dinv1_t[:], dinv1_d[:])
            recip_t = cp.tile([1, NSHP], dt.bfloat16, tag="recip")
            nc.sync.dma_start(recip_t[:], recip_d[:])

            def make_oh1(kk):
                oh = ohp.tile([128, 128], dt.bfloat16, tag="ohb")
                nc.vector.tensor_scalar(
                    oh[:], iota_t[:], dstl_t[:, kk:kk + 1], None,
                    mybir.AluOpType.is_equal)
                return oh[:]

            def make_oh(kk):
                oh = ohp.tile([128, 64], dt.uint16, tag="oh16")
                nc.vector.tensor_scalar(
                    oh[:], iota64_t[:], dpair_t[:, kk:kk + 1],
                    dpar_t[:, kk:kk + 1],
                    mybir.AluOpType.is_equal, mybir.AluOpType.mult)
                return oh[:].bitcast(TAB8)

            def win_tile_slot(G, j, kk, tiles):
                """gather tile + slot for chunk kk of run (G, j)."""
                k0, _ = runs[(G, j)]
                w = (kk - k0) // WCHUNK
                return tiles[(G, j)][w], (kk - k0) % WCHUNK

            h3_writes = {j: [] for j in range(SLICES)}
            h3s_tiles = {}
            exps = {}

            # ================= layer 1 =================
            with (
                tc.tile_pool(name="l1ps", bufs=2, space="PSUM") as app1,
                tc.tile_pool(name="h2ps", bufs=1, space="PSUM") as hpp,
                tc.tile_pool(name="h3ps", bufs=1, space="PSUM") as tpp,
            ):
                gt_tiles = {}
                for G in range(NG):
                    # issue all gathers of this supergroup (both slices)
                    for j in range(SLICES):
                        k0, k1 = runs[(G, j)]
                        tiles = []
                        for o in range(k0, k1, WCHUNK):
                            m = min(WCHUNK, k1 - o)
                            gt = g1p.tile([128, WCHUNK, D], dt.bfloat16, tag="g1")
                            nc.gpsimd.dma_gather(
                                gt[:, 0:m, :], Xs_d[j][:],
                                idx_t[:, o * 8:(o + m) * 8],
                                num_idxs=m * 128, num_idxs_reg=m * 128,
                                elem_size=D)
                            tiles.append(gt)
                        gt_tiles[(G, j)] = tiles

                    # consume block-major: self term, then both slices' segs
                    for b in range(G * GS, (G + 1) * GS):
                        accA = app1.tile([128, 128], dt.float32,
                                         name=f"accA{b}", tag="accA")
                        accB = app1.tile([128, 128], dt.float32,
                                         name=f"accB{b}", tag="accB")
                        acc = [accA, accB]
                        segs = segs_by_block[b]
                        # self-loop: acc[k,n] += Xown[n,k] via identity rhs
                        for kc in range(KD):
                            nc.tensor.matmul(acc[kc][:], xo_t[:, b, kc, :],
                                             id_t[:], start=True,
                                             stop=(len(segs) == 0))
                        for i, (j, kk, col) in enumerate(segs):
                            gt, s = win_tile_slot(G, j, kk, gt_tiles)
                            ohap = make_oh1(col)
                            last = (i == len(segs) - 1)
                            for kc in range(KD):
                                nc.tensor.matmul(
                                    acc[kc][:],
                                    gt[:, s, kc * 128:(kc + 1) * 128],
                                    ohap, start=False, stop=last)

                        # evac cascade: AGG -> W1+b1 -> relu -> W2 -> *dinv^2
                        agg = evp.tile([128, KD, 128], dt.bfloat16, tag="agg")
                        nc.scalar.activation(agg[:, 0, :], accA[:],
                                             mybir.ActivationFunctionType.Copy)
                        nc.scalar.activation(agg[:, 1, :], accB[:],
                                             mybir.ActivationFunctionType.Copy)
                        h2 = hpp.tile([F1, 128], dt.float32, tag="h2")
                        for kc in range(KD):
                            nc.tensor.matmul(h2[:], W1_t[:, kc, :], agg[:, kc, :],
                                             start=(kc == 0), stop=False)
                        nc.tensor.matmul(h2[:], b1_t[:],
                                         recip_t[0:1, b * 128:(b + 1) * 128],
                                         start=False, stop=True)
                        h2s = evp.tile([F1, 128], dt.bfloat16, tag="h2s")
                        nc.scalar.activation(h2s[:], h2[:],
                                             mybir.ActivationFunctionType.Relu)
                        h3 = tpp.tile([128, F2], dt.float32, tag="h3")
                        nc.tensor.matmul(h3[:], h2s[:], W2_t[:],
                                         start=True, stop=True)
                        h3s = h3p.tile([128, F2], TAB8,
                                       name=f"h3s{b}", tag=f"h3s{b}")
                        nc.scalar.activation(h3s[:], h3[:],
                                             mybir.ActivationFunctionType.Copy,
                                             bias=0.0, scale=dinv2_t[:, b:b + 1])
                        h3s_tiles[b] = h3s
                        j_b = blk_slice(b)
                        r0 = b * 128 - BOUNDS[j_b]
                        w = nc.sync.dma_start(H3shs[j_b][r0:r0 + 128, :], h3s[:])
                        h3_writes[j_b].append(w)
                    del gt_tiles[(G, 0)], gt_tiles[(G, 1)]

                    # fire slice AllGather + expansion as soon as ready
                    for j in range(SLICES):
                        if (G + 1) * GS * 128 == BOUNDS[j + 1]:
                            cc = nc.gpsimd.collective_compute(
                                "AllGather", mybir.AluOpType.bypass,
                                replica_groups=[list(range(NCORES))],
                                ins=[H3shs[j][:]], outs=[H3tabs[j][:]])
                            for w in h3_writes[j]:
                                add_dep_helper(cc.ins, w.ins,
                                               reason="allgather reads H3 slice")
                            ex = nc.sync.dma_start(H3exp[j][:, 0:F2], H3tabs[j][:])
                            add_dep_helper(ex.ins, cc.ins,
                                           reason="expand reads allgathered tab")
                            exps[j] = ex

            # ================= layer 2 =================
            with tc.tile_pool(name="l2ps", bufs=3, space="PSUM") as app2:
                parks = {}
                for j in range(SLICES):
                    for G in range(NG):
                        k0, k1 = runs[(G, j)]
                        tiles = []
                        for o in range(k0, k1, WCHUNK):
                            m = min(WCHUNK, k1 - o)
                            gt8 = g2p.tile([128, WCHUNK, 256], TAB8, tag="g2")
                            gi = nc.gpsimd.dma_gather(
                                gt8[:, 0:m, :], H3exp[j][:],
                                idx_t[:, o * 8:(o + m) * 8],
                                num_idxs=m * 128, num_idxs_reg=m * 128,
                                elem_size=256, elem_step=256)
                            add_dep_helper(gi.ins, exps[j].ins,
                                           reason="gather reads expanded tab")
                            tiles.append(gt8)
                        gtt = {(G, j): tiles}

                        for b in range(G * GS, (G + 1) * GS):
                            segs = [t for t in segs_by_block[b] if t[0] == j]
                            m_j = len(segs)
                            if j == 0:
                                a = app2.tile([128, F2], dt.float32,
                                              name=f"acc2_{b}_0", tag="acc2")
                                # self-loop: acc2[n,f] += h3s[n,f]
                                nc.tensor.matmul(a[:], id_t[:],
                                                 h3s_tiles[b][:],
                                                 start=True, stop=(m_j == 0))
                                for i, (_, kk, col) in enumerate(segs):
                                    gt8, s = win_tile_slot(G, 0, kk, gtt)
                                    nc.tensor.matmul(
                                        a[:], make_oh(col), gt8[:, s, 0:F2],
                                        start=False, stop=(i == m_j - 1))
                                pk = pkp.tile([128, F2], dt.bfloat16,
                                              name=f"park{b}", tag=f"pk{b}")
                                nc.vector.tensor_copy(pk[:], a[:])
                                parks[b] = pk
                            else:
                                a = app2.tile([128, F2], dt.float32,
                                              name=f"acc2_{b}_1", tag="acc2")
                                for i, (_, kk, col) in enumerate(segs):
                                    gt8, s = win_tile_slot(G, 1, kk, gtt)
                                    nc.tensor.matmul(
                                        a[:], make_oh(col), gt8[:, s, 0:F2],
                                        start=(i == 0), stop=False)
                                # bias (b2 (x) sqrt(deg)) closes the group
                                nc.tensor.matmul(
                                    a[:], recip_t[0:1, b * 128:(b + 1) * 128],
                                    b2_t[:], start=(m_j == 0), stop=True)
                                tmp = evp.tile([128, F2], dt.float32, tag="tmp")
                                nc.vector.tensor_tensor(
                                    tmp[:], a[:], parks[b][:],
                                    mybir.AluOpType.add)
                                ost = evp.tile([128, F2], dt.float32, tag="ost")
                                nc.scalar.activation(
                                    ost[:], tmp[:],
                                    mybir.ActivationFunctionType.Copy,
                                    bias=0.0, scale=dinv1_t[:, b:b + 1])
                                nc.sync.dma_start(
                                    out_d[b * 128:(b + 1) * 128, :], ost[:])

    if not nc.is_finalized():
        nc.finalize()
    hoist_excess_waits(nc)
    return nc


# ---------------------------------------------------------------------------
def _kernel_impl(x, edge_index, W1, b1, W2, b2, ncores=NCORES):
    x = np.asarray(x, dtype=np.float32)
    edge_index = np.asarray(edge_index)
    W1 = np.asarray(W1, dtype=np.float32)
    b1 = np.asarray(b1, dtype=np.float32)
    W2 = np.asarray(W2, dtype=np.float32)
    b2 = np.asarray(b2, dtype=np.float32)
    N, D = x.shape
    F1 = W1.shape[1]
    F2 = W2.shape[1]

    cfg = _prepare(x, edge_index, ncores)
    nc = _build(cfg, F1, F2)

    bf16 = _np_dt(dt.bfloat16)
    in_maps = []
    for c in range(ncores):
        m = {f"Xs{j}": cfg['Xs'][j] for j in range(SLICES)}
        m.update({
            "Xown": cfg['Xown_np'][c],
            "W1": W1.astype(bf16),
            "W2": W2.astype(bf16),
            "b1": b1.reshape(1, F1).astype(bf16),
            "b2": b2.reshape(1, F2).astype(bf16),
            "iota64": cfg['iota64'],
            "iota": cfg['iota'].astype(bf16),
            "id128": cfg['id128'].astype(_np_dt(dt.float8e4)),
            "dstl": cfg['dstl_np'][c],
            "idx": cfg['idx_np'][c],
            "dpair": cfg['dpair_np'][c],
            "dpar": cfg['dpar_np'][c],
            "dinv2": cfg['dinv2_np'][c],
            "dinv1": cfg['dinv1_np'][c],
            "recip": cfg['recip_np'][c].astype(bf16),
        })
        in_maps.append(m)
    res = run_bass_kernel_spmd(nc, in_maps, list(range(ncores)))

    NSH = cfg['NSH']
    out = np.empty((N, F2), np.float32)
    for c in range(ncores):
        o = res.results[c]["out"]            # [NSHP, F2]
        n0 = c * NSH
        n1 = min(N, n0 + NSH)
        out[n0:n1] = o[:n1 - n0]
    return out, res, nc, cfg


def kernel(x, edge_index, W1, b1, W2, b2):
    out, _, _, _ = _kernel_impl(x, edge_index, W1, b1, W2, b2)
    return out


# revision 12
# speedup vs baseline: 1.6088x; 1.0038x over previous
"""Two-layer GCN (GCNConv x2 + ReLU) on 8 Trainium2 NeuronCores.

Strategy (aggregate-first, dinv-folded, fp8 tables):
  - Nodes sharded by destination across 8 cores. Layer 1 aggregates RAW
    input features: each core gathers X'[src] rows per edge (X' = dinv*X,
    host-prescaled, fp8, laid out in 2 slice tables), scatter-adds them into
    per-dst-block PSUM accumulators (feature-major) via pure 0/1 one-hot
    matmuls, then applies W1 + bias + ReLU + W2 per 128-node block on-chip.
    No X@W1 table phase at all.
  - dinv normalization folded exactly: one-hots are pure 0/1; source dinv
    lives in the tables; dst dinv is applied as a per-partition activation
    scale (relu is positively homogeneous): H3' table gets dinv^2, final
    output gets dinv. Biases enter as K=1 matmuls of b (x) sqrt(deg).
  - H3' = dinv^2*relu(AGG@W1 + b1*sqrt(deg))@W2 stored fp8 e4m3 at 64
    features -> the AllGather moves 3.2MB total instead of 12.8MB. After the
    AllGather each slice is expanded to 256B-row stride (dma_gather needs
    elem/stride multiples of 256B); layer 2 gathers those rows node-major.
  - One-hots are built once per 128-edge chunk as packed uint16 pairs on DVE
    (2-byte dtype keeps the 4x DVE mode, 77ns) and bitcast to fp8 [128,128].
  - Self-loops are excluded from the edge lists (that removes exactly one
    full chunk per (block, slice-of-own-rows)): their contribution enters
    via identity matmuls against directly-read own-shard X' rows (layer 1)
    and the layer-1 H3 evac tiles still in SBUF (layer 2).
  - Edge chunks grouped (supergroup of 7 dst blocks) x (source slice of 2)
    so one SWDGE dma_gather covers up to 4096 edges (scratch ring enlarged),
    amortizing the ~1.1us per-gather prep cost. Chunks are CONSUMED
    block-major so only ~2 blocks' PSUM accumulators are live.
"""
import sys
sys.path.insert(0, '/opt/trn_rl_repo')
import numpy as np
import concourse.bass as bass
import concourse.bacc as bacc
import concourse.mybir as mybir
import bass_rust
from concourse.tile import TileContext
from concourse.tile_rust import add_dep_helper
from concourse.bass_utils import run_bass_kernel_spmd

dt = mybir.dt

NCORES = 8
SLICES = 2
WCHUNK = 8            # chunks per gather window (1024 idx = HW SWDGE ring cap)
SCRATCH = 16384        # SWDGE ring: 1024 descriptors (HW cap)
TAB8 = dt.float8e4     # table dtype (e4m3)
PAD_DST = 999.0        # one-hot miss value for padding edges


def _np_dt(d):
    return mybir.dt.np(d)


# ---------------------------------------------------------------------------
# walrus in this toolchain rejects >1 attached sem wait on several opcodes;
# hoist extras into standalone InstEventSemaphore instructions just before.
def hoist_excess_waits(nc, max_attached=1):
    n_new = 0
    for f in nc.m.functions:
        for bb in f.blocks:
            insts = bb.instructions  # live list
            i = 0
            while i < len(insts):
                inst = insts[i]
                si = inst.sync_info
                if si is not None and inst.engine is not None:
                    waits = list(si.on_wait)
                    imm = [w for w in waits if w.wait_reg is None]
                    other = [w for w in waits if w.wait_reg is not None]
                    budget = max_attached - len(other)
                    if len(imm) > budget:
                        if budget > 0:
                            extra, keep = imm[:-budget], imm[-budget:]
                        else:
                            extra, keep = imm, []
                        for w in extra:
                            ev = mybir.InstEventSemaphore(
                                name=f"I-hoistw{n_new}", ins=[], outs=[])
                            ev.engine = inst.engine
                            h = bass_rust.SemaphoreHandle(name=w.ant_name, num=w.id)
                            bass_rust.wait_op(ev, h, w.wait_value, "sem-ge", True)
                            insts.insert(i, ev)
                            i += 1
                            n_new += 1
                        si.on_wait = other + keep
                i += 1
    return n_new


# ---------------------------------------------------------------------------
# node rebalance: permute nodes within each shard so that per-(block, slice)
# in-edge counts are flat across blocks (the shared SPMD chunk schedule is
# sized by the max over cores; flat per-core counts minimize that max).
def _rebalance(src, dst, N, NSH, TS, B1ROWS, ncores, iters=2):
    caps = np.full(TS, 128, np.int64)
    caps[TS - 1] = NSH - (TS - 1) * 128
    pos = np.arange(N, dtype=np.int64)           # node -> shard-local slot
    base = (np.arange(N) // NSH) * NSH
    for _ in range(iters):
        l = pos - base
        jlab = (l >= B1ROWS)
        d0 = np.bincount(dst[~jlab[src]], minlength=N).astype(np.float64)
        d1 = np.bincount(dst[jlab[src]], minlength=N).astype(np.float64)
        newl = np.empty(N, np.int64)
        for c in range(ncores):
            ids = np.arange(c * NSH, (c + 1) * NSH)
            dd0, dd1 = d0[ids], d1[ids]
            t0 = max(dd0.sum() / TS, 1.0)
            t1 = max(dd1.sum() / TS, 1.0)
            order = np.argsort(-(dd0 + dd1), kind='stable')
            s0 = np.zeros(TS)
            s1 = np.zeros(TS)
            cnt = np.zeros(TS, np.int64)
            assign = np.empty(NSH, np.int64)
            for i in order:
                cost = np.maximum((s0 + dd0[i]) / t0, (s1 + dd1[i]) / t1)
                cost[cnt >= caps] = np.inf
                bsel = int(np.argmin(cost))
                assign[i] = bsel
                s0[bsel] += dd0[i]
                s1[bsel] += dd1[i]
                cnt[bsel] += 1
            # slot within block: fill order
            fill = np.zeros(TS, np.int64)
            for i in range(NSH):
                b = assign[i]
                newl[c * NSH + i] = b * 128 + fill[b]
                fill[b] += 1
        pos = base + newl
    return pos


# ---------------------------------------------------------------------------
# host-side graph preprocessing
def _prepare(x, edge_index, ncores):
    x = np.asarray(x, dtype=np.float32)
    N, D = x.shape
    NSH = (N + ncores - 1) // ncores            # nodes per shard (6250)
    TS = (NSH + 127) // 128                     # dst blocks per shard (49)
    GS = max(s for s in range(1, 9) if TS % s == 0)   # blocks per supergroup
    NG = TS // GS                               # supergroups per shard
    NSHP = TS * 128                             # padded shard rows

    # slice bounds in shard rows (multiples of GS*128); near-even split
    gb, rem = NG // SLICES, NG % SLICES
    parts = [gb + (1 if i < rem else 0) for i in range(SLICES)]
    BOUNDS = [0]
    for p in parts:
        BOUNDS.append(BOUNDS[-1] + p * GS * 128)
    RSLS = [BOUNDS[i + 1] - BOUNDS[i] for i in range(SLICES)]
    assert all(ncores * r < 32768 for r in RSLS), RSLS
    BARR = np.array(BOUNDS)
    RARR = np.array(RSLS)

    src = edge_index[0].astype(np.int64)        # self-loops handled separately
    dst = edge_index[1].astype(np.int64)
    E = len(src)

    # permute nodes for balance; everything below works in the new id space
    newpos = _rebalance(src, dst, N, NSH, TS, BOUNDS[1], ncores)
    x = x[np.argsort(newpos)]
    src = newpos[src]
    dst = newpos[dst]

    deg = np.bincount(dst, minlength=N).astype(np.float32) + 1.0  # + self loop
    dinv = 1.0 / np.sqrt(deg)

    # source table row (slice-local): node s -> shard c, local l;
    # slice j of l; row = c*RSL_j + (l - B_j)
    src_c, src_l = src // NSH, src % NSH
    src_j = (np.searchsorted(BARR, src_l, side='right') - 1).astype(np.int64)
    src_row = src_c * RARR[src_j] + (src_l - BARR[src_j])

    dst_c, dst_l = dst // NSH, dst % NSH
    dst_b = dst_l // 128
    dst_p = dst_l % 128

    # cell = (core, block, slice); shared schedule sized by max count per cell
    cell = (dst_c * TS + dst_b) * SLICES + src_j
    counts = np.bincount(cell, minlength=ncores * TS * SLICES)
    counts3 = counts.reshape(ncores, TS, SLICES)
    mx_cnt = counts3.max(axis=0)                        # [TS, SLICES]

    # continuous packing: per run (G, j), cells' edge spans (length mx_cnt)
    # are laid back-to-back; chunks of 128 descs may straddle cells. Each
    # (cell, chunk) intersection is a SEGMENT with its own one-hot column.
    cell_off = np.zeros((TS, SLICES), np.int64)   # desc offset of cell
    runs = {}                                     # (G, j) -> (k0, k1) chunks
    seg_block = []                                # per segment: block
    segs_by_block = {b: [] for b in range(TS)}    # b -> [(j, kk, col)]
    first_seg_of_cell = np.zeros((TS, SLICES), np.int64)
    first_chunk_of_cell = np.zeros((TS, SLICES), np.int64)
    k = 0                                         # chunk counter
    for G in range(NG):
        for j in range(SLICES):
            k0 = k
            d = 0                                 # desc offset within run
            for b in range(G * GS, (G + 1) * GS):
                cnt = int(mx_cnt[b, j])
                cell_off[b, j] = k0 * 128 + d
                if cnt > 0:
                    first_seg_of_cell[b, j] = len(seg_block)
                    first_chunk_of_cell[b, j] = k0 + d // 128
                    for kk in range(k0 + d // 128,
                                    k0 + (d + cnt - 1) // 128 + 1):
                        col = len(seg_block)
                        seg_block.append(b)
                        segs_by_block[b].append((j, kk, col))
                d += cnt
            k = k0 + (d + 127) // 128
            runs[(G, j)] = (k0, k)
    NCHT = k
    NSEG = len(seg_block)

    # rank of each edge within its cell (stable order)
    order = np.argsort(cell, kind='stable')
    starts = np.zeros(ncores * TS * SLICES + 1, np.int64)
    starts[1:] = np.cumsum(counts)
    rank = np.empty(E, np.int64)
    rank[order] = np.arange(E) - starts[cell[order]]

    slot = cell_off[dst_b, src_j] + rank        # flat desc slot per edge
    edge_col = (first_seg_of_cell[dst_b, src_j]
                + slot // 128 - first_chunk_of_cell[dst_b, src_j])

    idx_np = np.zeros((ncores, 128, NCHT * 8), np.int16)
    dstl_np = np.full((ncores, 128, NSEG), PAD_DST, np.float32)
    dpair_np = np.full((ncores, 128, NSEG), PAD_DST, np.float32)
    dpar_np = np.zeros((ncores, 128, NSEG), np.float32)
    for c in range(ncores):
        m = dst_c == c
        fsrc = np.zeros(NCHT * 128, np.int64)
        fsrc[slot[m]] = src_row[m]
        i16 = fsrc.astype(np.int16).reshape(-1, 16).T      # [16, NCHT*8]
        idx_np[c] = np.tile(i16, (8, 1))
        p_m = slot[m] % 128
        col_m = edge_col[m]
        dstl_np[c][p_m, col_m] = dst_p[m]
        dpair_np[c][p_m, col_m] = np.floor(dst_p[m] / 2.0)
        dpar_np[c][p_m, col_m] = np.where(dst_p[m] % 2 == 0, 56.0, 14336.0)

    # X' tables per slice (bf16), row = c*RSL_j + (l - B_j); pad rows zero
    f8 = _np_dt(TAB8)
    bf16 = _np_dt(dt.bfloat16)
    xp = (x * dinv[:, None]).astype(np.float32)
    Xs = []
    for j in range(SLICES):
        t = np.zeros((ncores * RSLS[j], D), np.float32)
        for c in range(ncores):
            l0, l1 = BOUNDS[j], BOUNDS[j + 1]
            n0 = c * NSH + l0
            n1 = min(c * NSH + min(l1, NSH), N)
            if n1 > n0:
                t[c * RSLS[j]:c * RSLS[j] + (n1 - n0)] = xp[n0:n1]
        Xs.append(t.astype(bf16))

    # per-core own-shard X' rows (plain order) for the self-loop term
    Xown_np = np.zeros((ncores, NSHP, D), np.float32)
    for c in range(ncores):
        n0, n1 = c * NSH, min((c + 1) * NSH, N)
        Xown_np[c, :n1 - n0] = xp[n0:n1]
    Xown_np = Xown_np.astype(f8)

    # per-core dst-side scales
    dinv2_np = np.ones((ncores, 128, TS), np.float32)
    dinv1_np = np.ones((ncores, 128, TS), np.float32)
    recip_np = np.zeros((ncores, 1, NSHP), np.float32)
    for c in range(ncores):
        n0, n1 = c * NSH, min((c + 1) * NSH, N)
        dloc = np.ones(NSHP, np.float32)
        dloc[:n1 - n0] = dinv[n0:n1]
        dinv2_np[c] = (dloc ** 2).reshape(TS, 128).T
        dinv1_np[c] = dloc.reshape(TS, 128).T
        r = np.zeros(NSHP, np.float32)
        r[:n1 - n0] = 1.0 / dinv[n0:n1]
        recip_np[c, 0] = r

    iota64 = np.tile(np.arange(64, dtype=np.uint16)[None, :], (128, 1)).copy()
    iota = np.tile(np.arange(128, dtype=np.float32)[None, :], (128, 1)).copy()
    id128 = np.eye(128, dtype=np.float32)

    return dict(N=N, D=D, NSH=NSH, TS=TS, GS=GS, NG=NG, NSHP=NSHP,
                newpos=newpos,
                BOUNDS=BOUNDS, RSLS=RSLS, NCHT=NCHT, NSEG=NSEG, runs=runs,
                segs_by_block=segs_by_block,
                idx_np=idx_np, dstl_np=dstl_np, dpair_np=dpair_np,
                dpar_np=dpar_np, iota=iota,
                Xs=Xs, Xown_np=Xown_np, dinv2_np=dinv2_np,
                dinv1_np=dinv1_np, recip_np=recip_np,
                iota64=iota64, id128=id128)


# ---------------------------------------------------------------------------
def _build(cfg, F1, F2):
    D, TS, GS, NG = cfg['D'], cfg['TS'], cfg['GS'], cfg['NG']
    NSHP, NCHT, NSEG = cfg['NSHP'], cfg['NCHT'], cfg['NSEG']
    BOUNDS, RSLS = cfg['BOUNDS'], cfg['RSLS']
    runs = cfg['runs']
    segs_by_block = cfg['segs_by_block']
    KD = D // 128

    nc = bacc.Bacc(None, target_bir_lowering=False,
                   dynamic_dma_scratch_size=SCRATCH)
    Xs_d = [nc.declare_dram_parameter(f"Xs{j}", [NCORES * RSLS[j], D],
                                      dt.bfloat16, isOutput=False)
            for j in range(SLICES)]
    Xown_d = nc.declare_dram_parameter("Xown", [NSHP, D], TAB8, isOutput=False)
    W1_d = nc.declare_dram_parameter("W1", [D, F1], dt.bfloat16, isOutput=False)
    W2_d = nc.declare_dram_parameter("W2", [F1, F2], dt.bfloat16, isOutput=False)
    b1_d = nc.declare_dram_parameter("b1", [1, F1], dt.bfloat16, isOutput=False)
    b2_d = nc.declare_dram_parameter("b2", [1, F2], dt.bfloat16, isOutput=False)
    iota64_d = nc.declare_dram_parameter("iota64", [128, 64], dt.uint16, isOutput=False)
    iota_d = nc.declare_dram_parameter("iota", [128, 128], dt.bfloat16, isOutput=False)
    id128_d = nc.declare_dram_parameter("id128", [128, 128], TAB8, isOutput=False)
    dstl_d = nc.declare_dram_parameter("dstl", [128, NSEG], dt.float32, isOutput=False)
    idx_d = nc.declare_dram_parameter("idx", [128, NCHT * 8], dt.int16, isOutput=False)
    dpair_d = nc.declare_dram_parameter("dpair", [128, NSEG], dt.float32, isOutput=False)
    dpar_d = nc.declare_dram_parameter("dpar", [128, NSEG], dt.float32, isOutput=False)
    dinv2_d = nc.declare_dram_parameter("dinv2", [128, TS], dt.float32, isOutput=False)
    dinv1_d = nc.declare_dram_parameter("dinv1", [128, TS], dt.float32, isOutput=False)
    recip_d = nc.declare_dram_parameter("recip", [1, NSHP], dt.bfloat16, isOutput=False)
    out_d = nc.declare_dram_parameter("out", [NSHP, F2], dt.float32, isOutput=True)

    H3shs = [nc.dram_tensor(f"H3sh{j}", [RSLS[j], F2], TAB8)
             for j in range(SLICES)]
    H3tabs = [nc.dram_tensor(f"H3tab{j}", [NCORES * RSLS[j], F2], TAB8,
                             addr_space="Shared") for j in range(SLICES)]
    H3exp = [nc.dram_tensor(f"H3exp{j}", [NCORES * RSLS[j], 256], TAB8)
             for j in range(SLICES)]

    def blk_slice(b):
        return 0 if (b + 1) * 128 <= BOUNDS[1] else 1

    max_run = max(k1 - k0 for (k1, k0) in ((b, a) for (a, b) in runs.values()))
    wpr = -(-max_run // WCHUNK)          # windows per run

    with TileContext(nc) as tc:
        with (
            tc.tile_pool(name="const", bufs=1) as cp,
            tc.tile_pool(name="l1gt", bufs=2 * wpr + 2) as g1p,
            tc.tile_pool(name="l2gt", bufs=wpr + 2) as g2p,
            tc.tile_pool(name="oh16", bufs=10) as ohp,
            tc.tile_pool(name="evac", bufs=3) as evp,
            tc.tile_pool(name="h3sb", bufs=1) as h3p,
            tc.tile_pool(name="park", bufs=1) as pkp,
        ):
            # ---- constants / metadata resident in SBUF ----
            iota64_t = cp.tile([128, 64], dt.uint16, tag="iota64")
            nc.sync.dma_start(iota64_t[:], iota64_d[:])
            id_t = cp.tile([128, 128], TAB8, tag="id128")
            nc.sync.dma_start(id_t[:], id128_d[:])
            iota_t = cp.tile([128, 128], dt.bfloat16, tag="iota")
            nc.sync.dma_start(iota_t[:], iota_d[:])
            dstl_t = cp.tile([128, NSEG], dt.float32, tag="dstl")
            nc.sync.dma_start(dstl_t[:], dstl_d[:])
            W1_t = cp.tile([128, KD, F1], dt.bfloat16, tag="W1")
            nc.sync.dma_start(W1_t[:], W1_d[:].rearrange("(k p) f -> p k f", p=128))
            W2_t = cp.tile([F1, F2], dt.bfloat16, tag="W2")
            nc.sync.dma_start(W2_t[:], W2_d[:])
            b1_t = cp.tile([1, F1], dt.bfloat16, tag="b1")
            nc.sync.dma_start(b1_t[:], b1_d[:])
            b2_t = cp.tile([1, F2], dt.bfloat16, tag="b2")
            nc.sync.dma_start(b2_t[:], b2_d[:])
            xo_t = cp.tile([128, TS, KD, 128], TAB8, tag="Xown")
            nc.sync.dma_start(
                xo_t[:], Xown_d[:].rearrange("(t p) (k f) -> p t k f",
                                             p=128, k=KD))
            idx_t = cp.tile([128, NCHT * 8], dt.int16, tag="idx")
            nc.sync.dma_start(idx_t[:], idx_d[:])
            dpair_t = cp.tile([128, NSEG], dt.float32, tag="dpair")
            nc.sync.dma_start(dpair_t[:], dpair_d[:])
            dpar_t = cp.tile([128, NSEG], dt.float32, tag="dpar")
            nc.sync.dma_start(dpar_t[:], dpar_d[:])
            dinv2_t = cp.tile([128, TS], dt.float32, tag="dinv2")
            nc.sync.dma_start(dinv2_t[:], dinv2_d[:])
            dinv1_t = cp.tile([128, TS], dt.float32, tag="dinv1")
            nc.sync.dma_start(dinv1_t[:], dinv1_d[:])
            recip_t = cp.tile([1, NSHP], dt.bfloat16, tag="recip")
            nc.sync.dma_start(recip_t[:], recip_d[:])

            def make_oh1(kk):
                oh = ohp.tile([128, 128], dt.bfloat16, tag="ohb")
                nc.vector.tensor_scalar(
                    oh[:], iota_t[:], dstl_t[:, kk:kk + 1], None,
                    mybir.AluOpType.is_equal)
                return oh[:]

            def make_oh(kk):
                oh = ohp.tile([128, 64], dt.uint16, tag="oh16")
                nc.vector.tensor_scalar(
                    oh[:], iota64_t[:], dpair_t[:, kk:kk + 1],
                    dpar_t[:, kk:kk + 1],
                    mybir.AluOpType.is_equal, mybir.AluOpType.mult)
                return oh[:].bitcast(TAB8)

            def win_tile_slot(G, j, kk, tiles):
                """gather tile + slot for chunk kk of run (G, j)."""
                k0, _ = runs[(G, j)]
                w = (kk - k0) // WCHUNK
                return tiles[(G, j)][w], (kk - k0) % WCHUNK

            h3_writes = {j: [] for j in range(SLICES)}
            h3s_tiles = {}
            exps = {}

            # ================= layer 1 =================
            with (
                tc.tile_pool(name="l1ps", bufs=2, space="PSUM") as app1,
                tc.tile_pool(name="h2ps", bufs=1, space="PSUM") as hpp,
                tc.tile_pool(name="h3ps", bufs=1, space="PSUM") as tpp,
            ):
                gt_tiles = {}
                for G in range(NG):
                    # issue all gathers of this supergroup (both slices)
                    for j in range(SLICES):
                        k0, k1 = runs[(G, j)]
                        tiles = []
                        for o in range(k0, k1, WCHUNK):
                            m = min(WCHUNK, k1 - o)
                            gt = g1p.tile([128, WCHUNK, D], dt.bfloat16, tag="g1")
                            nc.gpsimd.dma_gather(
                                gt[:, 0:m, :], Xs_d[j][:],
                                idx_t[:, o * 8:(o + m) * 8],
                                num_idxs=m * 128, num_idxs_reg=m * 128,
                                elem_size=D)
                            tiles.append(gt)
                        gt_tiles[(G, j)] = tiles

                    # consume block-major: self term, then both slices' segs
                    for b in range(G * GS, (G + 1) * GS):
                        accA = app1.tile([128, 128], dt.float32,
                                         name=f"accA{b}", tag="accA")
                        accB = app1.tile([128, 128], dt.float32,
                                         name=f"accB{b}", tag="accB")
                        acc = [accA, accB]
                        segs = segs_by_block[b]
                        # self-loop: acc[k,n] += Xown[n,k] via identity rhs
                        for kc in range(KD):
                            nc.tensor.matmul(acc[kc][:], xo_t[:, b, kc, :],
                                             id_t[:], start=True,
                                             stop=(len(segs) == 0))
                        for i, (j, kk, col) in enumerate(segs):
                            gt, s = win_tile_slot(G, j, kk, gt_tiles)
                            ohap = make_oh1(col)
                            last = (i == len(segs) - 1)
                            for kc in range(KD):
                                nc.tensor.matmul(
                                    acc[kc][:],
                                    gt[:, s, kc * 128:(kc + 1) * 128],
                                    ohap, start=False, stop=last)

                        # evac cascade: AGG -> W1+b1 -> relu -> W2 -> *dinv^2
                        agg = evp.tile([128, KD, 128], dt.bfloat16, tag="agg")
                        nc.scalar.activation(agg[:, 0, :], accA[:],
                                             mybir.ActivationFunctionType.Copy)
                        nc.scalar.activation(agg[:, 1, :], accB[:],
                                             mybir.ActivationFunctionType.Copy)
                        h2 = hpp.tile([F1, 128], dt.float32, tag="h2")
                        for kc in range(KD):
                            nc.tensor.matmul(h2[:], W1_t[:, kc, :], agg[:, kc, :],
                                             start=(kc == 0), stop=False)
                        nc.tensor.matmul(h2[:], b1_t[:],
                                         recip_t[0:1, b * 128:(b + 1) * 128],
                                         start=False, stop=True)
                        h2s = evp.tile([F1, 128], dt.bfloat16, tag="h2s")
                        nc.scalar.activation(h2s[:], h2[:],
                                             mybir.ActivationFunctionType.Relu)
                        h3 = tpp.tile([128, F2], dt.float32, tag="h3")
                        nc.tensor.matmul(h3[:], h2s[:], W2_t[:],
                                         start=True, stop=True)
                        h3s = h3p.tile([128, F2], TAB8,
                                       name=f"h3s{b}", tag=f"h3s{b}")
                        nc.scalar.activation(h3s[:], h3[:],
                                             mybir.ActivationFunctionType.Copy,
                                             bias=0.0, scale=dinv2_t[:, b:b + 1])
                        h3s_tiles[b] = h3s
                        j_b = blk_slice(b)
                        r0 = b * 128 - BOUNDS[j_b]
                        w = nc.sync.dma_start(H3shs[j_b][r0:r0 + 128, :], h3s[:])
                        h3_writes[j_b].append(w)
                    del gt_tiles[(G, 0)], gt_tiles[(G, 1)]

                    # fire slice AllGather + expansion as soon as ready
                    for j in range(SLICES):
                        if (G + 1) * GS * 128 == BOUNDS[j + 1]:
                            cc = nc.gpsimd.collective_compute(
                                "AllGather", mybir.AluOpType.bypass,
                                replica_groups=[list(range(NCORES))],
                                ins=[H3shs[j][:]], outs=[H3tabs[j][:]])
                            for w in h3_writes[j]:
                                add_dep_helper(cc.ins, w.ins,
                                               reason="allgather reads H3 slice")
                            ex = nc.sync.dma_start(H3exp[j][:, 0:F2], H3tabs[j][:])
                            add_dep_helper(ex.ins, cc.ins,
                                           reason="expand reads allgathered tab")
                            exps[j] = ex

            # ================= layer 2 =================
            with tc.tile_pool(name="l2ps", bufs=3, space="PSUM") as app2:
                parks = {}
                for j in range(SLICES):
                    for G in range(NG):
                        k0, k1 = runs[(G, j)]
                        tiles = []
                        for o in range(k0, k1, WCHUNK):
                            m = min(WCHUNK, k1 - o)
                            gt8 = g2p.tile([128, WCHUNK, 256], TAB8, tag="g2")
                            gi = nc.gpsimd.dma_gather(
                                gt8[:, 0:m, :], H3exp[j][:],
                                idx_t[:, o * 8:(o + m) * 8],
                                num_idxs=m * 128, num_idxs_reg=m * 128,
                                elem_size=256, elem_step=256)
                            add_dep_helper(gi.ins, exps[j].ins,
                                           reason="gather reads expanded tab")
                            tiles.append(gt8)
                        gtt = {(G, j): tiles}

                        for b in range(G * GS, (G + 1) * GS):
                            segs = [t for t in segs_by_block[b] if t[0] == j]
                            m_j = len(segs)
                            if j == 0:
                                a = app2.tile([128, F2], dt.float32,
                                              name=f"acc2_{b}_0", tag="acc2")
                                # self-loop: acc2[n,f] += h3s[n,f]
                                nc.tensor.matmul(a[:], id_t[:],
                                                 h3s_tiles[b][:],
                                                 start=True, stop=(m_j == 0))
                                for i, (_, kk, col) in enumerate(segs):
                                    gt8, s = win_tile_slot(G, 0, kk, gtt)
                                    nc.tensor.matmul(
                                        a[:], make_oh(col), gt8[:, s, 0:F2],
                                        start=False, stop=(i == m_j - 1))
                                pk = pkp.tile([128, F2], dt.bfloat16,
                                              name=f"park{b}", tag=f"pk{b}")
                                nc.vector.tensor_copy(pk[:], a[:])
                                parks[b] = pk
                            else:
                                a = app2.tile([128, F2], dt.float32,
                                              name=f"acc2_{b}_1", tag="acc2")
                                for i, (_, kk, col) in enumerate(segs):
                                    gt8, s = win_tile_slot(G, 1, kk, gtt)
                                    nc.tensor.matmul(
                                        a[:], make_oh(col), gt8[:, s, 0:F2],
                                        start=(i == 0), stop=False)
                                # bias (b2 (x) sqrt(deg)) closes the group
                                nc.tensor.matmul(
                                    a[:], recip_t[0:1, b * 128:(b + 1) * 128],
                                    b2_t[:], start=(m_j == 0), stop=True)
                                tmp = evp.tile([128, F2], dt.float32, tag="tmp")
                                nc.vector.tensor_tensor(
                                    tmp[:], a[:], parks[b][:],
                                    mybir.AluOpType.add)
                                ost = evp.tile([128, F2], dt.float32, tag="ost")
                                nc.scalar.activation(
                                    ost[:], tmp[:],
                                    mybir.ActivationFunctionType.Copy,
                                    bias=0.0, scale=dinv1_t[:, b:b + 1])
                                nc.sync.dma_start(
                                    out_d[b * 128:(b + 1) * 128, :], ost[:])

    if not nc.is_finalized():
        nc.finalize()
    hoist_excess_waits(nc)
    return nc


# ---------------------------------------------------------------------------
def _kernel_impl(x, edge_index, W1, b1, W2, b2, ncores=NCORES):
    x = np.asarray(x, dtype=np.float32)
    edge_index = np.asarray(edge_index)
    W1 = np.asarray(W1, dtype=np.float32)
    b1 = np.asarray(b1, dtype=np.float32)
    W2 = np.asarray(W2, dtype=np.float32)
    b2 = np.asarray(b2, dtype=np.float32)
    N, D = x.shape
    F1 = W1.shape[1]
    F2 = W2.shape[1]

    cfg = _prepare(x, edge_index, ncores)
    nc = _build(cfg, F1, F2)

    bf16 = _np_dt(dt.bfloat16)
    in_maps = []
    for c in range(ncores):
        m = {f"Xs{j}": cfg['Xs'][j] for j in range(SLICES)}
        m.update({
            "Xown": cfg['Xown_np'][c],
            "W1": W1.astype(bf16),
            "W2": W2.astype(bf16),
            "b1": b1.reshape(1, F1).astype(bf16),
            "b2": b2.reshape(1, F2).astype(bf16),
            "iota64": cfg['iota64'],
            "iota": cfg['iota'].astype(bf16),
            "id128": cfg['id128'].astype(_np_dt(dt.float8e4)),
            "dstl": cfg['dstl_np'][c],
            "idx": cfg['idx_np'][c],
            "dpair": cfg['dpair_np'][c],
            "dpar": cfg['dpar_np'][c],
            "dinv2": cfg['dinv2_np'][c],
            "dinv1": cfg['dinv1_np'][c],
            "recip": cfg['recip_np'][c].astype(bf16),
        })
        in_maps.append(m)
    res = run_bass_kernel_spmd(nc, in_maps, list(range(ncores)))

    NSH = cfg['NSH']
    outp = np.empty((N, F2), np.float32)
    for c in range(ncores):
        o = res.results[c]["out"]            # [NSHP, F2]
        n0 = c * NSH
        n1 = min(N, n0 + NSH)
        outp[n0:n1] = o[:n1 - n0]
    out = outp[cfg['newpos']]                # back to original node order
    return out, res, nc, cfg


def kernel(x, edge_index, W1, b1, W2, b2):
    out, _, _, _ = _kernel_impl(x, edge_index, W1, b1, W2, b2)
    return out


# revision 13
# speedup vs baseline: 1.6393x; 1.0190x over previous
"""Two-layer GCN (GCNConv x2 + ReLU) on 8 Trainium2 NeuronCores.

Strategy (aggregate-first, dinv-folded, fp8 tables):
  - Nodes sharded by destination across 8 cores. Layer 1 aggregates RAW
    input features: each core gathers X'[src] rows per edge (X' = dinv*X,
    host-prescaled, fp8, laid out in 2 slice tables), scatter-adds them into
    per-dst-block PSUM accumulators (feature-major) via pure 0/1 one-hot
    matmuls, then applies W1 + bias + ReLU + W2 per 128-node block on-chip.
    No X@W1 table phase at all.
  - dinv normalization folded exactly: one-hots are pure 0/1; source dinv
    lives in the tables; dst dinv is applied as a per-partition activation
    scale (relu is positively homogeneous): H3' table gets dinv^2, final
    output gets dinv. Biases enter as K=1 matmuls of b (x) sqrt(deg).
  - H3' = dinv^2*relu(AGG@W1 + b1*sqrt(deg))@W2 stored fp8 e4m3 at 64
    features -> the AllGather moves 3.2MB total instead of 12.8MB. After the
    AllGather each slice is expanded to 256B-row stride (dma_gather needs
    elem/stride multiples of 256B); layer 2 gathers those rows node-major.
  - One-hots are built once per 128-edge chunk as packed uint16 pairs on DVE
    (2-byte dtype keeps the 4x DVE mode, 77ns) and bitcast to fp8 [128,128].
  - Self-loops are excluded from the edge lists (that removes exactly one
    full chunk per (block, slice-of-own-rows)): their contribution enters
    via identity matmuls against directly-read own-shard X' rows (layer 1)
    and the layer-1 H3 evac tiles still in SBUF (layer 2).
  - Edge chunks grouped (supergroup of 7 dst blocks) x (source slice of 2)
    so one SWDGE dma_gather covers up to 4096 edges (scratch ring enlarged),
    amortizing the ~1.1us per-gather prep cost. Chunks are CONSUMED
    block-major so only ~2 blocks' PSUM accumulators are live.
"""
import sys
sys.path.insert(0, '/opt/trn_rl_repo')
import numpy as np
import concourse.bass as bass
import concourse.bacc as bacc
import concourse.mybir as mybir
import bass_rust
from concourse.tile import TileContext
from concourse.tile_rust import add_dep_helper
from concourse.bass_utils import run_bass_kernel_spmd

dt = mybir.dt

NCORES = 8
SLICES = 2
WCHUNK = 8            # chunks per gather window (1024 idx = HW SWDGE ring cap)
SCRATCH = 16384        # SWDGE ring: 1024 descriptors (HW cap)
TAB8 = dt.float8e4     # table dtype (e4m3)
PAD_DST = 999.0        # one-hot miss value for padding edges


def _np_dt(d):
    return mybir.dt.np(d)


# ---------------------------------------------------------------------------
# walrus in this toolchain rejects >1 attached sem wait on several opcodes;
# hoist extras into standalone InstEventSemaphore instructions just before.
def hoist_excess_waits(nc, max_attached=1):
    n_new = 0
    for f in nc.m.functions:
        for bb in f.blocks:
            insts = bb.instructions  # live list
            i = 0
            while i < len(insts):
                inst = insts[i]
                si = inst.sync_info
                if si is not None and inst.engine is not None:
                    waits = list(si.on_wait)
                    imm = [w for w in waits if w.wait_reg is None]
                    other = [w for w in waits if w.wait_reg is not None]
                    budget = max_attached - len(other)
                    if len(imm) > budget:
                        if budget > 0:
                            extra, keep = imm[:-budget], imm[-budget:]
                        else:
                            extra, keep = imm, []
                        for w in extra:
                            ev = mybir.InstEventSemaphore(
                                name=f"I-hoistw{n_new}", ins=[], outs=[])
                            ev.engine = inst.engine
                            h = bass_rust.SemaphoreHandle(name=w.ant_name, num=w.id)
                            bass_rust.wait_op(ev, h, w.wait_value, "sem-ge", True)
                            insts.insert(i, ev)
                            i += 1
                            n_new += 1
                        si.on_wait = other + keep
                i += 1
    return n_new


# ---------------------------------------------------------------------------
# node rebalance: permute nodes within each shard so that per-(block, slice)
# in-edge counts are flat across blocks (the shared SPMD chunk schedule is
# sized by the max over cores; flat per-core counts minimize that max).
def _rebalance(src, dst, N, NSH, TS, B1ROWS, ncores):
    caps = np.full(TS, 128, np.int64)
    caps[TS - 1] = NSH - (TS - 1) * 128
    B1B = B1ROWS // 128                          # blocks in slice 0
    l_nat = np.arange(N, dtype=np.int64) % NSH
    jmem = l_nat >= B1ROWS                       # fixed slice membership
    d0 = np.bincount(dst[~jmem[src]], minlength=N).astype(np.float64)
    d1 = np.bincount(dst[jmem[src]], minlength=N).astype(np.float64)
    pos = np.empty(N, np.int64)
    for c in range(ncores):
        for jm, blocks in ((False, np.arange(B1B)),
                           (True, np.arange(B1B, TS))):
            ids = np.nonzero((np.arange(N) // NSH == c) & (jmem == jm))[0]
            dd0, dd1 = d0[ids], d1[ids]
            nb = len(blocks)
            t0 = max(dd0.sum() / nb, 1.0)
            t1 = max(dd1.sum() / nb, 1.0)
            order = np.argsort(-(dd0 + dd1), kind='stable')
            s0 = np.zeros(nb)
            s1 = np.zeros(nb)
            cnt = np.zeros(nb, np.int64)
            bcaps = caps[blocks]
            for i in order:
                cost = np.maximum((s0 + dd0[i]) / t0, (s1 + dd1[i]) / t1)
                cost[cnt >= bcaps] = np.inf
                bsel = int(np.argmin(cost))
                s0[bsel] += dd0[i]
                s1[bsel] += dd1[i]
                pos[ids[i]] = c * NSH + (blocks[bsel] * 128 + cnt[bsel])
                cnt[bsel] += 1
    return pos


# ---------------------------------------------------------------------------
# host-side graph preprocessing
def _prepare(x, edge_index, ncores):
    x = np.asarray(x, dtype=np.float32)
    N, D = x.shape
    NSH = (N + ncores - 1) // ncores            # nodes per shard (6250)
    TS = (NSH + 127) // 128                     # dst blocks per shard (49)
    GS = max(s for s in range(1, 9) if TS % s == 0)   # blocks per supergroup
    NG = TS // GS                               # supergroups per shard
    NSHP = TS * 128                             # padded shard rows

    # slice bounds in shard rows (multiples of GS*128); near-even split
    gb, rem = NG // SLICES, NG % SLICES
    parts = [gb + (1 if i < rem else 0) for i in range(SLICES)]
    BOUNDS = [0]
    for p in parts:
        BOUNDS.append(BOUNDS[-1] + p * GS * 128)
    RSLS = [BOUNDS[i + 1] - BOUNDS[i] for i in range(SLICES)]
    assert all(ncores * r < 32768 for r in RSLS), RSLS
    BARR = np.array(BOUNDS)
    RARR = np.array(RSLS)

    src = edge_index[0].astype(np.int64)        # self-loops handled separately
    dst = edge_index[1].astype(np.int64)
    E = len(src)

    # permute nodes for balance; everything below works in the new id space
    newpos = _rebalance(src, dst, N, NSH, TS, BOUNDS[1], ncores)
    x = x[np.argsort(newpos)]
    src = newpos[src]
    dst = newpos[dst]

    deg = np.bincount(dst, minlength=N).astype(np.float32) + 1.0  # + self loop
    dinv = 1.0 / np.sqrt(deg)

    # source table row (slice-local): node s -> shard c, local l;
    # slice j of l; row = c*RSL_j + (l - B_j)
    src_c, src_l = src // NSH, src % NSH
    src_j = (np.searchsorted(BARR, src_l, side='right') - 1).astype(np.int64)
    src_row = src_c * RARR[src_j] + (src_l - BARR[src_j])

    dst_c, dst_l = dst // NSH, dst % NSH
    dst_b = dst_l // 128
    dst_p = dst_l % 128

    # cell = (core, block, slice); shared schedule sized by max count per cell
    cell = (dst_c * TS + dst_b) * SLICES + src_j
    counts = np.bincount(cell, minlength=ncores * TS * SLICES)
    counts3 = counts.reshape(ncores, TS, SLICES)
    mx_cnt = counts3.max(axis=0)                        # [TS, SLICES]

    # continuous packing: per run (G, j), cells' edge spans (length mx_cnt)
    # are laid back-to-back; chunks of 128 descs may straddle cells. Each
    # (cell, chunk) intersection is a SEGMENT with its own one-hot column.
    cell_off = np.zeros((TS, SLICES), np.int64)   # desc offset of cell
    runs = {}                                     # (G, j) -> (k0, k1) chunks
    seg_block = []                                # per segment: block
    segs_by_block = {b: [] for b in range(TS)}    # b -> [(j, kk, col)]
    first_seg_of_cell = np.zeros((TS, SLICES), np.int64)
    first_chunk_of_cell = np.zeros((TS, SLICES), np.int64)
    k = 0                                         # chunk counter
    for G in range(NG):
        for j in range(SLICES):
            k0 = k
            d = 0                                 # desc offset within run
            for b in range(G * GS, (G + 1) * GS):
                cnt = int(mx_cnt[b, j])
                cell_off[b, j] = k0 * 128 + d
                if cnt > 0:
                    first_seg_of_cell[b, j] = len(seg_block)
                    first_chunk_of_cell[b, j] = k0 + d // 128
                    for kk in range(k0 + d // 128,
                                    k0 + (d + cnt - 1) // 128 + 1):
                        col = len(seg_block)
                        seg_block.append(b)
                        segs_by_block[b].append((j, kk, col))
                d += cnt
            k = k0 + (d + 127) // 128
            runs[(G, j)] = (k0, k)
    NCHT = k
    NSEG = len(seg_block)

    # rank of each edge within its cell (stable order)
    order = np.argsort(cell, kind='stable')
    starts = np.zeros(ncores * TS * SLICES + 1, np.int64)
    starts[1:] = np.cumsum(counts)
    rank = np.empty(E, np.int64)
    rank[order] = np.arange(E) - starts[cell[order]]

    slot = cell_off[dst_b, src_j] + rank        # flat desc slot per edge
    edge_col = (first_seg_of_cell[dst_b, src_j]
                + slot // 128 - first_chunk_of_cell[dst_b, src_j])

    idx_np = np.zeros((ncores, 128, NCHT * 8), np.int16)
    dstl_np = np.full((ncores, 128, NSEG), PAD_DST, np.float32)
    dpair_np = np.full((ncores, 128, NSEG), PAD_DST, np.float32)
    dpar_np = np.zeros((ncores, 128, NSEG), np.float32)
    for c in range(ncores):
        m = dst_c == c
        fsrc = np.zeros(NCHT * 128, np.int64)
        fsrc[slot[m]] = src_row[m]
        i16 = fsrc.astype(np.int16).reshape(-1, 16).T      # [16, NCHT*8]
        idx_np[c] = np.tile(i16, (8, 1))
        p_m = slot[m] % 128
        col_m = edge_col[m]
        dstl_np[c][p_m, col_m] = dst_p[m]
        dpair_np[c][p_m, col_m] = np.floor(dst_p[m] / 2.0)
        dpar_np[c][p_m, col_m] = np.where(dst_p[m] % 2 == 0, 56.0, 14336.0)

    # X' tables per slice (bf16), row = c*RSL_j + (l - B_j); pad rows zero
    f8 = _np_dt(TAB8)
    bf16 = _np_dt(dt.bfloat16)
    xp = (x * dinv[:, None]).astype(np.float32)
    Xs = []
    for j in range(SLICES):
        t = np.zeros((ncores * RSLS[j], D), np.float32)
        for c in range(ncores):
            l0, l1 = BOUNDS[j], BOUNDS[j + 1]
            n0 = c * NSH + l0
            n1 = min(c * NSH + min(l1, NSH), N)
            if n1 > n0:
                t[c * RSLS[j]:c * RSLS[j] + (n1 - n0)] = xp[n0:n1]
        Xs.append(t.astype(bf16))

    # per-core own-shard X' rows (plain order) for the self-loop term
    Xown_np = np.zeros((ncores, NSHP, D), np.float32)
    for c in range(ncores):
        n0, n1 = c * NSH, min((c + 1) * NSH, N)
        Xown_np[c, :n1 - n0] = xp[n0:n1]
    Xown_np = Xown_np.astype(f8)

    # per-core dst-side scales
    dinv2_np = np.ones((ncores, 128, TS), np.float32)
    dinv1_np = np.ones((ncores, 128, TS), np.float32)
    recip_np = np.zeros((ncores, 1, NSHP), np.float32)
    for c in range(ncores):
        n0, n1 = c * NSH, min((c + 1) * NSH, N)
        dloc = np.ones(NSHP, np.float32)
        dloc[:n1 - n0] = dinv[n0:n1]
        dinv2_np[c] = (dloc ** 2).reshape(TS, 128).T
        dinv1_np[c] = dloc.reshape(TS, 128).T
        r = np.zeros(NSHP, np.float32)
        r[:n1 - n0] = 1.0 / dinv[n0:n1]
        recip_np[c, 0] = r

    iota64 = np.tile(np.arange(64, dtype=np.uint16)[None, :], (128, 1)).copy()
    iota = np.tile(np.arange(128, dtype=np.float32)[None, :], (128, 1)).copy()
    id128 = np.eye(128, dtype=np.float32)

    return dict(N=N, D=D, NSH=NSH, TS=TS, GS=GS, NG=NG, NSHP=NSHP,
                newpos=newpos,
                BOUNDS=BOUNDS, RSLS=RSLS, NCHT=NCHT, NSEG=NSEG, runs=runs,
                segs_by_block=segs_by_block,
                idx_np=idx_np, dstl_np=dstl_np, dpair_np=dpair_np,
                dpar_np=dpar_np, iota=iota,
                Xs=Xs, Xown_np=Xown_np, dinv2_np=dinv2_np,
                dinv1_np=dinv1_np, recip_np=recip_np,
                iota64=iota64, id128=id128)


# ---------------------------------------------------------------------------
def _build(cfg, F1, F2):
    D, TS, GS, NG = cfg['D'], cfg['TS'], cfg['GS'], cfg['NG']
    NSHP, NCHT, NSEG = cfg['NSHP'], cfg['NCHT'], cfg['NSEG']
    BOUNDS, RSLS = cfg['BOUNDS'], cfg['RSLS']
    runs = cfg['runs']
    segs_by_block = cfg['segs_by_block']
    KD = D // 128

    nc = bacc.Bacc(None, target_bir_lowering=False,
                   dynamic_dma_scratch_size=SCRATCH)
    Xs_d = [nc.declare_dram_parameter(f"Xs{j}", [NCORES * RSLS[j], D],
                                      dt.bfloat16, isOutput=False)
            for j in range(SLICES)]
    Xown_d = nc.declare_dram_parameter("Xown", [NSHP, D], TAB8, isOutput=False)
    W1_d = nc.declare_dram_parameter("W1", [D, F1], dt.bfloat16, isOutput=False)
    W2_d = nc.declare_dram_parameter("W2", [F1, F2], dt.bfloat16, isOutput=False)
    b1_d = nc.declare_dram_parameter("b1", [1, F1], dt.bfloat16, isOutput=False)
    b2_d = nc.declare_dram_parameter("b2", [1, F2], dt.bfloat16, isOutput=False)
    iota64_d = nc.declare_dram_parameter("iota64", [128, 64], dt.uint16, isOutput=False)
    iota_d = nc.declare_dram_parameter("iota", [128, 128], dt.bfloat16, isOutput=False)
    id128_d = nc.declare_dram_parameter("id128", [128, 128], TAB8, isOutput=False)
    dstl_d = nc.declare_dram_parameter("dstl", [128, NSEG], dt.float32, isOutput=False)
    idx_d = nc.declare_dram_parameter("idx", [128, NCHT * 8], dt.int16, isOutput=False)
    dpair_d = nc.declare_dram_parameter("dpair", [128, NSEG], dt.float32, isOutput=False)
    dpar_d = nc.declare_dram_parameter("dpar", [128, NSEG], dt.float32, isOutput=False)
    dinv2_d = nc.declare_dram_parameter("dinv2", [128, TS], dt.float32, isOutput=False)
    dinv1_d = nc.declare_dram_parameter("dinv1", [128, TS], dt.float32, isOutput=False)
    recip_d = nc.declare_dram_parameter("recip", [1, NSHP], dt.bfloat16, isOutput=False)
    out_d = nc.declare_dram_parameter("out", [NSHP, F2], dt.float32, isOutput=True)

    H3shs = [nc.dram_tensor(f"H3sh{j}", [RSLS[j], F2], TAB8)
             for j in range(SLICES)]
    H3tabs = [nc.dram_tensor(f"H3tab{j}", [NCORES * RSLS[j], F2], TAB8,
                             addr_space="Shared") for j in range(SLICES)]
    H3exp = [nc.dram_tensor(f"H3exp{j}", [NCORES * RSLS[j], 256], TAB8)
             for j in range(SLICES)]

    def blk_slice(b):
        return 0 if (b + 1) * 128 <= BOUNDS[1] else 1

    max_run = max(k1 - k0 for (k1, k0) in ((b, a) for (a, b) in runs.values()))
    wpr = -(-max_run // WCHUNK)          # windows per run

    with TileContext(nc) as tc:
        with (
            tc.tile_pool(name="const", bufs=1) as cp,
            tc.tile_pool(name="l1gt", bufs=2 * wpr + 2) as g1p,
            tc.tile_pool(name="l2gt", bufs=wpr + 2) as g2p,
            tc.tile_pool(name="oh16", bufs=10) as ohp,
            tc.tile_pool(name="evac", bufs=3) as evp,
            tc.tile_pool(name="h3sb", bufs=1) as h3p,
            tc.tile_pool(name="park", bufs=1) as pkp,
        ):
            # ---- constants / metadata resident in SBUF ----
            iota64_t = cp.tile([128, 64], dt.uint16, tag="iota64")
            nc.sync.dma_start(iota64_t[:], iota64_d[:])
            id_t = cp.tile([128, 128], TAB8, tag="id128")
            nc.sync.dma_start(id_t[:], id128_d[:])
            iota_t = cp.tile([128, 128], dt.bfloat16, tag="iota")
            nc.sync.dma_start(iota_t[:], iota_d[:])
            dstl_t = cp.tile([128, NSEG], dt.float32, tag="dstl")
            nc.sync.dma_start(dstl_t[:], dstl_d[:])
            W1_t = cp.tile([128, KD, F1], dt.bfloat16, tag="W1")
            nc.sync.dma_start(W1_t[:], W1_d[:].rearrange("(k p) f -> p k f", p=128))
            W2_t = cp.tile([F1, F2], dt.bfloat16, tag="W2")
            nc.sync.dma_start(W2_t[:], W2_d[:])
            b1_t = cp.tile([1, F1], dt.bfloat16, tag="b1")
            nc.sync.dma_start(b1_t[:], b1_d[:])
            b2_t = cp.tile([1, F2], dt.bfloat16, tag="b2")
            nc.sync.dma_start(b2_t[:], b2_d[:])
            xo_t = cp.tile([128, TS, KD, 128], TAB8, tag="Xown")
            nc.sync.dma_start(
                xo_t[:], Xown_d[:].rearrange("(t p) (k f) -> p t k f",
                                             p=128, k=KD))
            idx_t = cp.tile([128, NCHT * 8], dt.int16, tag="idx")
            nc.sync.dma_start(idx_t[:], idx_d[:])
            dpair_t = cp.tile([128, NSEG], dt.float32, tag="dpair")
            nc.sync.dma_start(dpair_t[:], dpair_d[:])
            dpar_t = cp.tile([128, NSEG], dt.float32, tag="dpar")
            nc.sync.dma_start(dpar_t[:], dpar_d[:])
            dinv2_t = cp.tile([128, TS], dt.float32, tag="dinv2")
            nc.sync.dma_start(dinv2_t[:], dinv2_d[:])
            dinv1_t = cp.tile([128, TS], dt.float32, tag="dinv1")
            nc.sync.dma_start(dinv1_t[:], dinv1_d[:])
            recip_t = cp.tile([1, NSHP], dt.bfloat16, tag="recip")
            nc.sync.dma_start(recip_t[:], recip_d[:])

            def make_oh1(kk):
                oh = ohp.tile([128, 128], dt.bfloat16, tag="ohb")
                nc.vector.tensor_scalar(
                    oh[:], iota_t[:], dstl_t[:, kk:kk + 1], None,
                    mybir.AluOpType.is_equal)
                return oh[:]

            def make_oh(kk):
                oh = ohp.tile([128, 64], dt.uint16, tag="oh16")
                nc.vector.tensor_scalar(
                    oh[:], iota64_t[:], dpair_t[:, kk:kk + 1],
                    dpar_t[:, kk:kk + 1],
                    mybir.AluOpType.is_equal, mybir.AluOpType.mult)
                return oh[:].bitcast(TAB8)

            def win_tile_slot(G, j, kk, tiles):
                """gather tile + slot for chunk kk of run (G, j)."""
                k0, _ = runs[(G, j)]
                w = (kk - k0) // WCHUNK
                return tiles[(G, j)][w], (kk - k0) % WCHUNK

            h3_writes = {j: [] for j in range(SLICES)}
            h3s_tiles = {}
            exps = {}

            # ================= layer 1 =================
            with (
                tc.tile_pool(name="l1ps", bufs=2, space="PSUM") as app1,
                tc.tile_pool(name="h2ps", bufs=1, space="PSUM") as hpp,
                tc.tile_pool(name="h3ps", bufs=1, space="PSUM") as tpp,
            ):
                gt_tiles = {}
                for G in range(NG):
                    # issue all gathers of this supergroup (both slices)
                    for j in range(SLICES):
                        k0, k1 = runs[(G, j)]
                        tiles = []
                        for o in range(k0, k1, WCHUNK):
                            m = min(WCHUNK, k1 - o)
                            gt = g1p.tile([128, WCHUNK, D], dt.bfloat16, tag="g1")
                            nc.gpsimd.dma_gather(
                                gt[:, 0:m, :], Xs_d[j][:],
                                idx_t[:, o * 8:(o + m) * 8],
                                num_idxs=m * 128, num_idxs_reg=m * 128,
                                elem_size=D)
                            tiles.append(gt)
                        gt_tiles[(G, j)] = tiles

                    # consume block-major: self term, then both slices' segs
                    for b in range(G * GS, (G + 1) * GS):
                        accA = app1.tile([128, 128], dt.float32,
                                         name=f"accA{b}", tag="accA")
                        accB = app1.tile([128, 128], dt.float32,
                                         name=f"accB{b}", tag="accB")
                        acc = [accA, accB]
                        segs = segs_by_block[b]
                        # self-loop: acc[k,n] += Xown[n,k] via identity rhs
                        for kc in range(KD):
                            nc.tensor.matmul(acc[kc][:], xo_t[:, b, kc, :],
                                             id_t[:], start=True,
                                             stop=(len(segs) == 0))
                        for i, (j, kk, col) in enumerate(segs):
                            gt, s = win_tile_slot(G, j, kk, gt_tiles)
                            ohap = make_oh1(col)
                            last = (i == len(segs) - 1)
                            for kc in range(KD):
                                nc.tensor.matmul(
                                    acc[kc][:],
                                    gt[:, s, kc * 128:(kc + 1) * 128],
                                    ohap, start=False, stop=last)

                        # evac cascade: AGG -> W1+b1 -> relu -> W2 -> *dinv^2
                        agg = evp.tile([128, KD, 128], dt.bfloat16, tag="agg")
                        nc.scalar.activation(agg[:, 0, :], accA[:],
                                             mybir.ActivationFunctionType.Copy)
                        nc.scalar.activation(agg[:, 1, :], accB[:],
                                             mybir.ActivationFunctionType.Copy)
                        h2 = hpp.tile([F1, 128], dt.float32, tag="h2")
                        for kc in range(KD):
                            nc.tensor.matmul(h2[:], W1_t[:, kc, :], agg[:, kc, :],
                                             start=(kc == 0), stop=False)
                        nc.tensor.matmul(h2[:], b1_t[:],
                                         recip_t[0:1, b * 128:(b + 1) * 128],
                                         start=False, stop=True)
                        h2s = evp.tile([F1, 128], dt.bfloat16, tag="h2s")
                        nc.scalar.activation(h2s[:], h2[:],
                                             mybir.ActivationFunctionType.Relu)
                        h3 = tpp.tile([128, F2], dt.float32, tag="h3")
                        nc.tensor.matmul(h3[:], h2s[:], W2_t[:],
                                         start=True, stop=True)
                        h3s = h3p.tile([128, F2], TAB8,
                                       name=f"h3s{b}", tag=f"h3s{b}")
                        nc.scalar.activation(h3s[:], h3[:],
                                             mybir.ActivationFunctionType.Copy,
                                             bias=0.0, scale=dinv2_t[:, b:b + 1])
                        h3s_tiles[b] = h3s
                        j_b = blk_slice(b)
                        r0 = b * 128 - BOUNDS[j_b]
                        w = nc.sync.dma_start(H3shs[j_b][r0:r0 + 128, :], h3s[:])
                        h3_writes[j_b].append(w)
                    del gt_tiles[(G, 0)], gt_tiles[(G, 1)]

                    # fire slice AllGather + expansion as soon as ready
                    for j in range(SLICES):
                        if (G + 1) * GS * 128 == BOUNDS[j + 1]:
                            cc = nc.gpsimd.collective_compute(
                                "AllGather", mybir.AluOpType.bypass,
                                replica_groups=[list(range(NCORES))],
                                ins=[H3shs[j][:]], outs=[H3tabs[j][:]])
                            for w in h3_writes[j]:
                                add_dep_helper(cc.ins, w.ins,
                                               reason="allgather reads H3 slice")
                            ex = nc.sync.dma_start(H3exp[j][:, 0:F2], H3tabs[j][:])
                            add_dep_helper(ex.ins, cc.ins,
                                           reason="expand reads allgathered tab")
                            exps[j] = ex

            # ================= layer 2 =================
            with tc.tile_pool(name="l2ps", bufs=3, space="PSUM") as app2:
                parks = {}
                for j in range(SLICES):
                    for G in range(NG):
                        k0, k1 = runs[(G, j)]
                        tiles = []
                        for o in range(k0, k1, WCHUNK):
                            m = min(WCHUNK, k1 - o)
                            gt8 = g2p.tile([128, WCHUNK, 256], TAB8, tag="g2")
                            gi = nc.gpsimd.dma_gather(
                                gt8[:, 0:m, :], H3exp[j][:],
                                idx_t[:, o * 8:(o + m) * 8],
                                num_idxs=m * 128, num_idxs_reg=m * 128,
                                elem_size=256, elem_step=256)
                            add_dep_helper(gi.ins, exps[j].ins,
                                           reason="gather reads expanded tab")
                            tiles.append(gt8)
                        gtt = {(G, j): tiles}

                        for b in range(G * GS, (G + 1) * GS):
                            segs = [t for t in segs_by_block[b] if t[0] == j]
                            m_j = len(segs)
                            if j == 0:
                                a = app2.tile([128, F2], dt.float32,
                                              name=f"acc2_{b}_0", tag="acc2")
                                # self-loop: acc2[n,f] += h3s[n,f]
                                nc.tensor.matmul(a[:], id_t[:],
                                                 h3s_tiles[b][:],
                                                 start=True, stop=(m_j == 0))
                                for i, (_, kk, col) in enumerate(segs):
                                    gt8, s = win_tile_slot(G, 0, kk, gtt)
                                    nc.tensor.matmul(
                                        a[:], make_oh(col), gt8[:, s, 0:F2],
                                        start=False, stop=(i == m_j - 1))
                                pk = pkp.tile([128, F2], dt.bfloat16,
                                              name=f"park{b}", tag=f"pk{b}")
                                nc.vector.tensor_copy(pk[:], a[:])
                                parks[b] = pk
                            else:
                                a = app2.tile([128, F2], dt.float32,
                                              name=f"acc2_{b}_1", tag="acc2")
                                for i, (_, kk, col) in enumerate(segs):
                                    gt8, s = win_tile_slot(G, 1, kk, gtt)
                                    nc.tensor.matmul(
                                        a[:], make_oh(col), gt8[:, s, 0:F2],
                                        start=(i == 0), stop=False)
                                # bias (b2 (x) sqrt(deg)) closes the group
                                nc.tensor.matmul(
                                    a[:], recip_t[0:1, b * 128:(b + 1) * 128],
                                    b2_t[:], start=(m_j == 0), stop=True)
                                tmp = evp.tile([128, F2], dt.float32, tag="tmp")
                                nc.vector.tensor_tensor(
                                    tmp[:], a[:], parks[b][:],
                                    mybir.AluOpType.add)
                                ost = evp.tile([128, F2], dt.float32, tag="ost")
                                nc.scalar.activation(
                                    ost[:], tmp[:],
                                    mybir.ActivationFunctionType.Copy,
                                    bias=0.0, scale=dinv1_t[:, b:b + 1])
                                nc.sync.dma_start(
                                    out_d[b * 128:(b + 1) * 128, :], ost[:])

    if not nc.is_finalized():
        nc.finalize()
    hoist_excess_waits(nc)
    return nc


# ---------------------------------------------------------------------------
def _kernel_impl(x, edge_index, W1, b1, W2, b2, ncores=NCORES):
    x = np.asarray(x, dtype=np.float32)
    edge_index = np.asarray(edge_index)
    W1 = np.asarray(W1, dtype=np.float32)
    b1 = np.asarray(b1, dtype=np.float32)
    W2 = np.asarray(W2, dtype=np.float32)
    b2 = np.asarray(b2, dtype=np.float32)
    N, D = x.shape
    F1 = W1.shape[1]
    F2 = W2.shape[1]

    cfg = _prepare(x, edge_index, ncores)
    nc = _build(cfg, F1, F2)

    bf16 = _np_dt(dt.bfloat16)
    in_maps = []
    for c in range(ncores):
        m = {f"Xs{j}": cfg['Xs'][j] for j in range(SLICES)}
        m.update({
            "Xown": cfg['Xown_np'][c],
            "W1": W1.astype(bf16),
            "W2": W2.astype(bf16),
            "b1": b1.reshape(1, F1).astype(bf16),
            "b2": b2.reshape(1, F2).astype(bf16),
            "iota64": cfg['iota64'],
            "iota": cfg['iota'].astype(bf16),
            "id128": cfg['id128'].astype(_np_dt(dt.float8e4)),
            "dstl": cfg['dstl_np'][c],
            "idx": cfg['idx_np'][c],
            "dpair": cfg['dpair_np'][c],
            "dpar": cfg['dpar_np'][c],
            "dinv2": cfg['dinv2_np'][c],
            "dinv1": cfg['dinv1_np'][c],
            "recip": cfg['recip_np'][c].astype(bf16),
        })
        in_maps.append(m)
    res = run_bass_kernel_spmd(nc, in_maps, list(range(ncores)))

    NSH = cfg['NSH']
    outp = np.empty((N, F2), np.float32)
    for c in range(ncores):
        o = res.results[c]["out"]            # [NSHP, F2]
        n0 = c * NSH
        n1 = min(N, n0 + NSH)
        outp[n0:n1] = o[:n1 - n0]
    out = outp[cfg['newpos']]                # back to original node order
    return out, res, nc, cfg


def kernel(x, edge_index, W1, b1, W2, b2):
    out, _, _, _ = _kernel_impl(x, edge_index, W1, b1, W2, b2)
    return out
